# revision 1
# baseline (speedup 1.0000x reference)
"""Self-contained Trainium2 (Bass) kernel for a 3-conv GCN encoder.

reference math (PyG GCNConv with edge weights, symmetric norm, self loops):
    deg[t]  = 1 + sum_{e: col[e]=t} ew[e]
    dinv    = deg ** -0.5
    agg(X)[t] = dinv[t] * ( dinv[t]*X[t] + sum_{e->t} ew[e]*dinv[src]*X[src] )
    h  = relu(agg(x) @ W1 + b1)            ->  hs := dinv * h
    mu = agg(h) @ Wmu + bmu ; logstd = agg(h) @ Wls + bls

Distribution: nodes target-sharded across 8 cores. Per-edge source rows are
fetched with int16 `dma_gather` from a replicated table (AllGather of the
per-shard scaled features xs = dinv*x, then hs). The int16 limit (32767) is
handled by splitting the table into NR row-ranges; per range, targets are
re-compacted and degree-sorted so slot-major grids have ~zero padding, and
per-range partial sums are re-merged by a tiny int16-clean gather pass.
"""

import numpy as np


def _make_cfg(n, e, p, nr, f=128, h=128, o=64, cols_per_call=16, mb_batch=8,
              grid_bufs=3, mg_bufs=2, work_bufs=4):
    sh = n // p
    nb = -(-sh // 128)
    npad = nb * 128
    tbl = p * npad
    assert tbl % nr == 0
    rs = tbl // nr
    assert rs <= 32600, (rs, "int16 gather range too large")
    return dict(n=n, e=e, p=p, f=f, h=h, o=o, sh=sh, nb=nb, npad=npad,
                tbl=tbl, nr=nr, rs=rs, cols_per_call=cols_per_call,
                mb_batch=mb_batch, grid_bufs=grid_bufs, mg_bufs=mg_bufs,
                work_bufs=work_bufs)


CFG_PROD = dict(n=100000, e=1600000, p=8, nr=4)


# ----------------------------------------------------------------------------
# walrus compat shim: this env's walrus rejects >1 sync-wait per instruction
# (and any wait on InstDrain); hoist excess waits onto InstEventSemaphore.
# ----------------------------------------------------------------------------

def _split_excess_waits(nc, max_inline=1):
    import concourse.mybir as mybir
    n_moved = 0
    for fn in nc.m.functions:
        for bb in fn.blocks:
            new_insts = []
            for inst in bb.instructions:
                si = inst.sync_info
                if si is not None and si.on_wait:
                    keep = 0 if isinstance(inst, mybir.InstDrain) else max_inline
                    if isinstance(inst, mybir.InstEventSemaphore):
                        keep = max(keep, 1)
                    waits = list(si.on_wait)
                    if len(waits) > keep:
                        hoist = waits[:-keep] if keep else waits
                        inline = waits[-keep:] if keep else []
                        for w in hoist:
                            ev = mybir.InstEventSemaphore(
                                name=nc.get_next_instruction_name(), ins=[], outs=[])
                            ev.engine = inst.engine
                            ev.sync_info = mybir.SyncInfo(on_wait=[w], on_update=[])
                            new_insts.append(ev)
                            n_moved += 1
                        si.on_wait = inline
                new_insts.append(inst)
            bb.instructions[:] = new_insts
    return n_moved


# ----------------------------------------------------------------------------
# host preprocessing (pure index/shuffle work; all FP math stays on device)
# ----------------------------------------------------------------------------

def _wrap16(idxs):
    """int16 index stream -> [128, n/16] tile (16-wrapped, 8x replicated)."""
    n = len(idxs)
    assert n % 16 == 0
    t = np.zeros((128, n // 16), dtype=np.int16)
    blk = idxs.reshape(n // 16, 16).T.astype(np.int16)
    for k in range(8):
        t[16 * k:16 * (k + 1), :] = blk
    return t


def _slot_ranks(sorted_keys):
    """for a sorted int array, rank of each element within its value-group."""
    n = len(sorted_keys)
    if n == 0:
        return np.zeros(0, dtype=np.int64)
    starts = np.r_[0, np.flatnonzero(np.diff(sorted_keys)) + 1]
    group_start = np.repeat(starts, np.diff(np.r_[starts, n]))
    return np.arange(n) - group_start


def _preprocess(cfg, x, edge_index, edge_attr, W1, b1, Wmu, bmu, Wls, bls):
    p = cfg["p"]
    sh, nb, npad, nr, rs = cfg["sh"], cfg["nb"], cfg["npad"], cfg["nr"], cfg["rs"]

    row = np.asarray(edge_index[0], dtype=np.int64)
    col = np.asarray(edge_index[1], dtype=np.int64)
    ew = np.asarray(edge_attr, dtype=np.float32)
    x = np.asarray(x, dtype=np.float32)

    # per-shard target permutation (by total in-degree, desc) -------------
    shard_of = col // sh
    tloc = col - shard_of * sh
    pis, poss = [], []
    for c in range(p):
        deg_cnt = np.bincount(tloc[shard_of == c], minlength=sh)
        pi = np.argsort(-deg_cnt, kind="stable")
        pi_full = np.concatenate([pi, np.arange(sh, npad)])
        pos = np.empty(npad, dtype=np.int64)
        pos[pi_full] = np.arange(npad)
        pis.append(pi_full)
        poss.append(pos)

    # table row of each edge's source (shard-major, per-shard pi order)
    src_shard = row // sh
    src_loc = row - src_shard * sh
    xrow = np.empty(len(row), dtype=np.int64)
    for c in range(p):
        m = src_shard == c
        xrow[m] = c * npad + poss[c][src_loc[m]]
    rng_of = xrow // rs
    lidx = xrow - rng_of * rs

    per_core = []
    for c in range(p):
        m = shard_of == c
        per_core.append(dict(q=poss[c][tloc[m]], r=rng_of[m],
                             li=lidx[m], ew=ew[m]))

    # per-(core, range) compact ordering; uniformized shapes --------------
    ncb = np.ones(nr, dtype=np.int64)
    Ls = [[None] * nr for _ in range(p)]
    orders = [[None] * nr for _ in range(p)]
    cpos = [[None] * nr for _ in range(p)]
    for c in range(p):
        pc = per_core[c]
        for r in range(nr):
            L = np.bincount(pc["q"][pc["r"] == r], minlength=npad)
            order = np.argsort(-L, kind="stable")
            cp = np.empty(npad, dtype=np.int64)
            cp[order] = np.arange(npad)
            Ls[c][r], orders[c][r], cpos[c][r] = L, order, cp
            nnz = int((L > 0).sum())
            ncb[r] = max(ncb[r], max(1, -(-nnz // 128)))

    S = [np.zeros(int(ncb[r]), dtype=np.int64) for r in range(nr)]
    for r in range(nr):
        for c in range(p):
            Lsort = Ls[c][r][orders[c][r]]
            for cb in range(int(ncb[r])):
                blk = Lsort[cb * 128:(cb + 1) * 128]
                if len(blk):
                    S[r][cb] = max(S[r][cb], int(blk.max()))
    gofs_r = np.concatenate([[0], np.cumsum([int(S[r].sum()) for r in range(nr)])])
    gcols = int(gofs_r[-1])

    TS = np.zeros(nb, dtype=np.int64)
    for c in range(p):
        Lt = np.bincount(per_core[c]["q"], minlength=npad)
        for b in range(nb):
            TS[b] = max(TS[b], int(Lt[b * 128:(b + 1) * 128].max()))
    tcols = max(1, int(TS.sum()))

    # per-core device arrays ---------------------------------------------
    in_maps = []
    wcat = np.concatenate([np.asarray(Wmu, np.float32),
                           np.asarray(Wls, np.float32)], axis=1)
    bcat = np.concatenate([np.asarray(bmu, np.float32),
                           np.asarray(bls, np.float32)])
    ident = np.eye(128, dtype=np.float32)
    colofs = [np.concatenate([[0], np.cumsum(S[r])]) for r in range(nr)]
    tofs = np.concatenate([[0], np.cumsum(TS)])

    for c in range(p):
        pc = per_core[c]
        ew_grid = np.zeros((128, max(1, gcols)), dtype=np.float32)
        gidx = np.zeros(max(128, gcols * 128), dtype=np.int64)
        for r in range(nr):
            mr = pc["r"] == r
            cq = cpos[c][r][pc["q"][mr]]
            lis, ews = pc["li"][mr], pc["ew"][mr]
            o = np.argsort(cq, kind="stable")
            cq_s, li_s, ew_s = cq[o], lis[o], ews[o]
            slot = _slot_ranks(cq_s)
            cb = cq_s // 128
            part = cq_s % 128
            gcol = gofs_r[r] + colofs[r][cb] + slot
            ew_grid[part, gcol] = ew_s
            gidx[gcol * 128 + part] = li_s

        ew_tgrid = np.zeros((128, tcols), dtype=np.float32)
        qs = pc["q"]
        o = np.argsort(qs, kind="stable")
        q_s, ew_s = qs[o], pc["ew"][o]
        slot = _slot_ranks(q_s)
        ew_tgrid[q_s % 128, tofs[q_s // 128] + slot] = ew_s

        mw = np.zeros((128, nr * nb), dtype=np.float32)
        midx = np.zeros(nr * nb * 128, dtype=np.int64)
        qq = np.arange(npad)
        for r in range(nr):
            present = Ls[c][r] > 0
            mw[qq % 128, r * nb + qq // 128] = present.astype(np.float32)
            midx[r * nb * 128 + qq] = np.where(present, cpos[c][r], 0)

        x_own = np.zeros((npad, 128), dtype=np.float32)
        x_own[:sh] = x[c * sh + pis[c][:sh]]

        in_maps.append({
            "x_own": x_own,
            "ew_grid": ew_grid,
            "ew_tgrid": ew_tgrid,
            "gidx": _wrap16(gidx),
            "midx": _wrap16(midx),
            "mw": mw,
            "w1": np.asarray(W1, np.float32),
            "wcat": wcat,
            "b1row": np.asarray(b1, np.float32).reshape(1, -1),
            "bcatrow": bcat.reshape(1, -1),
            "ident": ident,
        })

    meta = dict(ncb=[int(v) for v in ncb],
                S=[list(map(int, S[r])) for r in range(nr)],
                TS=list(map(int, TS)), gcols=max(1, gcols), tcols=tcols,
                gofs_r=list(map(int, gofs_r)))
    return in_maps, meta, pis


# ----------------------------------------------------------------------------
# device program
# ----------------------------------------------------------------------------

def _build(cfg, meta, split=True):
    import concourse.bacc as bacc
    import concourse.mybir as mybir
    from concourse.tile import TileContext

    p, h, o = cfg["p"], cfg["h"], cfg["o"]
    nb, npad, nr, rs = cfg["nb"], cfg["npad"], cfg["nr"], cfg["rs"]
    tbl = cfg["tbl"]
    ncb, S, TS = meta["ncb"], meta["S"], meta["TS"]
    gcols, tcols, gofs_r = meta["gcols"], meta["tcols"], meta["gofs_r"]
    CPC, MBB = cfg["cols_per_call"], cfg["mb_batch"]
    f32, i16 = mybir.dt.float32, mybir.dt.int16
    AX = mybir.AxisListType.X
    OP = mybir.AluOpType
    ACTF = mybir.ActivationFunctionType

    nc = bacc.Bacc(num_devices=p)
    ew_grid = nc.declare_dram_parameter("ew_grid", [128, gcols], f32, isOutput=False)
    ew_tgrid = nc.declare_dram_parameter("ew_tgrid", [128, tcols], f32, isOutput=False)
    x_own = nc.declare_dram_parameter("x_own", [npad, 128], f32, isOutput=False)
    gidx = nc.declare_dram_parameter("gidx", [128, gcols * 8], i16, isOutput=False)
    midx = nc.declare_dram_parameter("midx", [128, nr * nb * 8], i16, isOutput=False)
    mw = nc.declare_dram_parameter("mw", [128, nr * nb], f32, isOutput=False)
    w1 = nc.declare_dram_parameter("w1", [128, h], f32, isOutput=False)
    wcat = nc.declare_dram_parameter("wcat", [128, 2 * o], f32, isOutput=False)
    b1row = nc.declare_dram_parameter("b1row", [1, h], f32, isOutput=False)
    bcatrow = nc.declare_dram_parameter("bcatrow", [1, 2 * o], f32, isOutput=False)
    ident = nc.declare_dram_parameter("ident", [128, 128], f32, isOutput=False)
    out_ext = nc.declare_dram_parameter("out", [npad, 128], f32, isOutput=True)

    with TileContext(nc) as tc:
        with tc.tile_pool(name="dram", bufs=1, space="DRAM") as dram, \
             tc.tile_pool(name="persist", bufs=1) as pp, \
             tc.tile_pool(name="own", bufs=1) as ownp, \
             tc.tile_pool(name="gix", bufs=2) as gixp, \
             tc.tile_pool(name="grid", bufs=cfg["grid_bufs"]) as gp, \
             tc.tile_pool(name="mg", bufs=cfg["mg_bufs"]) as mgp, \
             tc.tile_pool(name="work", bufs=cfg["work_bufs"]) as wp, \
             tc.tile_pool(name="psum", bufs=4, space="PSUM") as psp:

            shard_t = dram.tile([npad, 128], f32, tag="shard")
            shared = "Shared" if p > 4 else "Local"
            full1_t = dram.tile([tbl, 128], f32, tag="full1", addr_space=shared)
            full2_t = dram.tile([tbl, 128], f32, tag="full2", addr_space=shared)
            partials = [dram.tile([ncb[r] * 128, 128], f32, tag=f"part{r}",
                                  name=f"part{r}") for r in range(nr)]

            ewg_t = pp.tile([128, gcols], f32, tag="ewg")
            midx_t = pp.tile([128, nr * nb * 8], i16, tag="midx")
            gix_cols = max(sum(S[r]) for r in range(nr)) * 8
            ewt_t = gixp.tile([128, tcols], f32, tag="gix", name="ewt_t")
            mw_t = pp.tile([128, nr * nb], f32, tag="mw")
            w1_t = pp.tile([128, h], f32, tag="w1")
            wcat_t = pp.tile([128, 2 * o], f32, tag="wcat")
            b1_t = pp.tile([1, h], f32, tag="b1")
            bcat_t = pp.tile([1, 2 * o], f32, tag="bcat")
            id_t = pp.tile([128, 128], f32, tag="id")
            ones_t = pp.tile([1, 128], f32, tag="ones")
            deg_t = pp.tile([128, nb], f32, tag="deg")
            d2_t = pp.tile([128, nb], f32, tag="d2")
            dv_t = pp.tile([128, nb], f32, tag="dv")
            dvrow_t = pp.tile([128, 128], f32, tag="dvrow")
            dvcat_t = pp.tile([1, nb * 128], f32, tag="dvcat")

            for t, src in [(ewg_t, ew_grid), (midx_t, midx), (mw_t, mw),
                           (w1_t, w1), (wcat_t, wcat), (b1_t, b1row),
                           (bcat_t, bcatrow), (id_t, ident)]:
                nc.sync.dma_start(out=t[:], in_=src[:])
            nc.sync.dma_start(out=ewt_t[:, :tcols], in_=ew_tgrid[:])
            nc.vector.memset(ones_t[:], 1.0)

            # deg -> 1/deg (= dinv^2) and dinv --------------------------------
            tof = 0
            for b in range(nb):
                if TS[b] > 0:
                    nc.vector.tensor_reduce(deg_t[:, b:b + 1],
                                            ewt_t[:, tof:tof + TS[b]],
                                            axis=AX, op=OP.add)
                else:
                    nc.vector.memset(deg_t[:, b:b + 1], 0.0)
                tof += TS[b]
            nc.vector.tensor_scalar_add(deg_t[:], deg_t[:], 1.0)
            nc.vector.reciprocal(d2_t[:], deg_t[:])
            nc.scalar.sqrt(dv_t[:], d2_t[:])
            # dinv rows at partition 0 (for the bias outer-product lhsT)
            dvr_ps = psp.tile([128, 128], f32, tag="ps")
            nc.tensor.transpose(dvr_ps[:nb, :], dv_t[:, :nb], id_t[:])
            nc.scalar.activation(dvrow_t[:nb, :], dvr_ps[:nb, :], ACTF.Copy)
            nc.sync.dma_start(
                out=dvcat_t[:].rearrange("p (b c) -> p b c", c=128)[0:1, :nb, :],
                in_=dvrow_t[:nb, :])

            # xs_own = dinv * x_own ; write shard -----------------------------
            own_tiles = []
            for b in range(nb):
                xt = ownp.tile([128, 128], f32, tag=f"own{b}")
                nc.sync.dma_start(out=xt[:], in_=x_own[b * 128:(b + 1) * 128, :])
                nc.vector.tensor_scalar_mul(xt[:], xt[:], dv_t[:, b:b + 1])
                nc.sync.dma_start(out=shard_t[b * 128:(b + 1) * 128, :], in_=xt[:])
                own_tiles.append(xt)

            groups = [list(range(p))]

            def allgather(dst):
                tc.strict_bb_all_engine_barrier()
                nc.gpsimd.collective_compute(
                    "AllGather", OP.bypass, replica_groups=groups,
                    ins=[shard_t.opt()], outs=[dst.opt()])
                tc.strict_bb_all_engine_barrier()

            # column -> compact-block map per range
            col2cb = []
            for r in range(nr):
                m = []
                for cb in range(ncb[r]):
                    m += [cb] * S[r][cb]
                col2cb.append(m)

            def layer(table, front_cols, wmat, bias_lhsT, bias_rhs, relu, out_writer):
                # aggregation into per-range partials
                for r in range(nr):
                    for cb in range(ncb[r]):
                        if S[r][cb] == 0:
                            zt = wp.tile([128, 128], f32, tag="pt")
                            nc.vector.memset(zt[:], 0.0)
                            nc.sync.dma_start(
                                out=partials[r][cb * 128:(cb + 1) * 128, :],
                                in_=zt[:])
                    total_cols = sum(S[r])
                    git = gixp.tile([128, gix_cols], i16, tag="gix",
                                    name=f"git_{r}")
                    nc.sync.dma_start(out=git[:, :total_cols * 8],
                                      in_=gidx[:, gofs_r[r] * 8:
                                               (gofs_r[r] + total_cols) * 8])
                    done = 0
                    pt = None
                    first = True
                    while done < total_cols:
                        ncall = min(CPC, total_cols - done)
                        grid = gp.tile([128, CPC * 128], f32, tag="grid")
                        nc.gpsimd.dma_gather(
                            out_ap=grid[:, :ncall * 128].rearrange(
                                "p (g c) -> p g c", c=128),
                            in_ap=table[r * rs:(r + 1) * rs, :],
                            idxs_ap=git[:, done * 8:(done + ncall) * 8],
                            num_idxs=ncall * 128, num_idxs_reg=ncall * 128,
                            elem_size=128, single_packet=False)
                        for j in range(ncall):
                            lcol = done + j
                            cb = col2cb[r][lcol]
                            gcol = gofs_r[r] + lcol
                            if pt is None:
                                pt = wp.tile([128, 128], f32, tag="pt")
                                first = True
                            src = grid[:, j * 128:(j + 1) * 128]
                            sc = ewg_t[:, gcol:gcol + 1]
                            if first:
                                nc.vector.tensor_scalar_mul(pt[:], src, sc)
                                first = False
                            else:
                                nc.vector.scalar_tensor_tensor(
                                    pt[:], src, sc, pt[:], OP.mult, OP.add)
                            last_of_cb = (lcol + 1 == total_cols
                                          or col2cb[r][lcol + 1] != cb)
                            if last_of_cb:
                                nc.sync.dma_start(
                                    out=partials[r][cb * 128:(cb + 1) * 128, :],
                                    in_=pt[:])
                                pt = None
                        done += ncall

                # merge + dense epilogue, batched over final blocks
                for b0 in range(0, nb, MBB):
                    nbb = min(MBB, nb - b0)
                    mgs = []
                    for r in range(nr):
                        mg = mgp.tile([128, MBB * 128], f32, tag=f"mg{r}")
                        s0 = (r * nb + b0) * 128
                        nc.gpsimd.dma_gather(
                            out_ap=mg[:, :nbb * 128].rearrange(
                                "p (g c) -> p g c", c=128),
                            in_ap=partials[r][:],
                            idxs_ap=midx_t[:, s0 // 16:(s0 + nbb * 128) // 16],
                            num_idxs=nbb * 128, num_idxs_reg=nbb * 128,
                            elem_size=128, single_packet=False)
                        mgs.append(mg)
                    for bi in range(nbb):
                        b = b0 + bi
                        agg = wp.tile([128, 128], f32, tag="agg")
                        prev = own_tiles[b]
                        for r in range(nr):
                            nc.vector.scalar_tensor_tensor(
                                agg[:], mgs[r][:, bi * 128:(bi + 1) * 128],
                                mw_t[:, r * nb + b:r * nb + b + 1],
                                prev[:], OP.mult, OP.add)
                            prev = agg
                        asc = wp.tile([128, 128], f32, tag="asc")
                        nc.scalar.activation(asc[:], agg[:], ACTF.Copy,
                                             scale=front_cols[:, b:b + 1])
                        tps = psp.tile([128, 128], f32, tag="ps")
                        nc.tensor.transpose(tps[:], asc[:], id_t[:])
                        aggT = wp.tile([128, 128], f32, tag="aggT")
                        nc.scalar.activation(aggT[:], tps[:], ACTF.Copy)
                        zps = psp.tile([128, 128], f32, tag="zps")
                        nc.tensor.matmul(zps[:], bias_lhsT(b), bias_rhs[:],
                                         start=True, stop=False)
                        nc.tensor.matmul(zps[:], aggT[:], wmat[:],
                                         start=False, stop=True)
                        res = wp.tile([128, 128], f32, tag="res")
                        nc.scalar.activation(res[:], zps[:],
                                             ACTF.Relu if relu else ACTF.Copy)
                        out_writer(b, res)

            # ---- layer 1: hs = relu(dinv^2*aggraw@W1 + dinv x b1) ----
            allgather(full1_t)

            def l1_write(b, res):
                nc.vector.tensor_copy(own_tiles[b][:], res[:])
                nc.sync.dma_start(out=shard_t[b * 128:(b + 1) * 128, :],
                                  in_=res[:])

            layer(full1_t, d2_t, w1_t,
                  lambda b: dvcat_t[:].rearrange("p (b c) -> p b c", c=128)[0:1, b, :],
                  b1_t, True, l1_write)

            # ---- layers 2+3: [mu|ls] = dinv*agg2raw@[Wmu|Wls] + [bmu|bls] ----
            allgather(full2_t)

            def l2_write(b, res):
                nc.sync.dma_start(out=out_ext[b * 128:(b + 1) * 128, :],
                                  in_=res[:])

            layer(full2_t, dv_t, wcat_t, lambda b: ones_t[:], bcat_t, False, l2_write)

    nc.finalize()
    if split:
        _split_excess_waits(nc)
    return nc


# ----------------------------------------------------------------------------
# top-level entry
# ----------------------------------------------------------------------------

_CACHE = {}


def get_built(cfg, meta):
    key = repr((sorted(cfg.items()), repr(meta)))
    if key not in _CACHE:
        _CACHE[key] = _build(cfg, meta)
    return _CACHE[key]


def run(inputs, cfg):
    from concourse.bass_utils import run_bass_kernel_spmd
    in_maps, meta, pis = _preprocess(cfg, **inputs)
    nc = get_built(cfg, meta)
    res = run_bass_kernel_spmd(nc, in_maps, list(range(cfg["p"])))
    return postprocess(res.results, pis, cfg)


def postprocess(results, pis, cfg):
    n, sh, o, p = cfg["n"], cfg["sh"], cfg["o"], cfg["p"]
    mu = np.empty((n, o), dtype=np.float32)
    ls = np.empty((n, o), dtype=np.float32)
    for c in range(p):
        out = results[c]["out"]
        pi = pis[c]
        real = pi < sh
        mu[c * sh + pi[real]] = out[real][:, :o]
        ls[c * sh + pi[real]] = out[real][:, o:2 * o]
    return mu, ls


def kernel(x, edge_index, edge_attr, W1, b1, Wmu, bmu, Wls, bls):
    cfg = _make_cfg(**CFG_PROD)
    return run(dict(x=x, edge_index=edge_index, edge_attr=edge_attr, W1=W1,
                    b1=b1, Wmu=Wmu, bmu=bmu, Wls=Wls, bls=bls), cfg)



# revision 3
# speedup vs baseline: 1.3101x; 1.3101x over previous
"""Self-contained Trainium2 (Bass) kernel for a 3-conv GCN encoder.

reference math (PyG GCNConv with edge weights, symmetric norm, self loops):
    deg[t]  = 1 + sum_{e: col[e]=t} ew[e]
    dinv    = deg ** -0.5
    agg(X)[t] = sum_{e->t, incl self w=1} ew[e] * X[src]   (X pre-scaled by dinv)
    h  = relu(d2 * agg(xs) @ W1 + dinv x b1)   ->  hs := relu(...) = dinv * h
    [mu|ls] = dinv * agg(hs) @ [Wmu|Wls] + [bmu|bls]

Distribution: nodes target-sharded across 8 cores, identity table order
(row of node v = shard*npad + local). Every core receives the FULL x and
builds the scaled gather tables locally (no feature AllGather for layer 1);
only a [npad,1] dinv column is AllGather'd. hs is AllGather'd once in bf16.
Edges (incl explicit self loops) are laid out in degree-sorted per-(range,
half) compact slot grids for int16 dma_gather; per-range partial sums are
merged per final block with a 4-way gather + 3 adds (absent entries hit a
dedicated zero row).
"""

import numpy as np


CFG = dict(n=100000, e=1600000, p=8, f=128, h=128, o=64,
           sh=12500, nb=98, npad=12544, tbl=100352, nr=4, rs=25088,
           hh=2, hblk=49, cpc=32, mbb=4, sgrp=14,
           grid_bufs=3, acc_bufs=4, mg_bufs=2, wp_bufs=4)


# ----------------------------------------------------------------------------
# walrus compat shim: this env's walrus rejects >1 sync-wait per instruction
# (and any wait on InstDrain); hoist excess waits onto InstEventSemaphore.
# ----------------------------------------------------------------------------

def _split_excess_waits(nc, max_inline=1):
    import concourse.mybir as mybir
    n_moved = 0
    for fn in nc.m.functions:
        for bb in fn.blocks:
            new_insts = []
            for inst in bb.instructions:
                si = inst.sync_info
                if si is not None and si.on_wait:
                    keep = 0 if isinstance(inst, mybir.InstDrain) else max_inline
                    if isinstance(inst, mybir.InstEventSemaphore):
                        keep = max(keep, 1)
                    waits = list(si.on_wait)
                    if len(waits) > keep:
                        hoist = waits[:-keep] if keep else waits
                        inline = waits[-keep:] if keep else []
                        for w in hoist:
                            ev = mybir.InstEventSemaphore(
                                name=nc.get_next_instruction_name(), ins=[], outs=[])
                            ev.engine = inst.engine
                            ev.sync_info = mybir.SyncInfo(on_wait=[w], on_update=[])
                            new_insts.append(ev)
                            n_moved += 1
                        si.on_wait = inline
                new_insts.append(inst)
            bb.instructions[:] = new_insts
    return n_moved


# ----------------------------------------------------------------------------
# host preprocessing (pure index/shuffle work; all FP math stays on device)
# ----------------------------------------------------------------------------

def _wrap16(idxs):
    """int16 index stream -> [128, n/16] tile (16-wrapped, 8x replicated)."""
    n = len(idxs)
    assert n % 16 == 0
    t = np.zeros((128, n // 16), dtype=np.int16)
    blk = idxs.reshape(n // 16, 16).T.astype(np.int16)
    for k in range(8):
        t[16 * k:16 * (k + 1), :] = blk
    return t


def _slot_ranks(sorted_keys):
    """for a sorted int array, rank of each element within its value-group."""
    n = len(sorted_keys)
    if n == 0:
        return np.zeros(0, dtype=np.int64)
    starts = np.r_[0, np.flatnonzero(np.diff(sorted_keys)) + 1]
    group_start = np.repeat(starts, np.diff(np.r_[starts, n]))
    return np.arange(n) - group_start


def _preprocess(cfg, x, edge_index, edge_attr, W1, b1, Wmu, bmu, Wls, bls):
    p, sh, nb, npad = cfg["p"], cfg["sh"], cfg["nb"], cfg["npad"]
    nr, rs, hh, hblk = cfg["nr"], cfg["rs"], cfg["hh"], cfg["hblk"]
    hsz = hblk * 128                      # final rows per half

    row = np.asarray(edge_index[0], dtype=np.int64)
    col = np.asarray(edge_index[1], dtype=np.int64)
    ew = np.asarray(edge_attr, dtype=np.float32)
    x = np.asarray(x, dtype=np.float32)

    # full x in (identity) table order, padded per shard ---------------------
    x_tab = np.zeros((cfg["tbl"], 128), dtype=np.float32)
    for c in range(p):
        x_tab[c * npad:c * npad + sh] = x[c * sh:(c + 1) * sh]

    ss = row // sh
    trow_all = ss * npad + (row - ss * sh)      # table row of each edge source
    tshard = col // sh

    # per (core, range, half) edge groups; uniformized compact shapes --------
    per = [[[None] * hh for _ in range(nr)] for _ in range(p)]
    ncb = np.zeros((nr, hh), dtype=np.int64)
    for c in range(p):
        m = tshard == c
        tq = col[m] - c * sh
        trow = trow_all[m]
        wts = ew[m]
        # self loops
        tq = np.concatenate([tq, np.arange(sh)])
        trow_s = c * npad + np.arange(sh)
        trow = np.concatenate([trow, trow_s])
        wts = np.concatenate([wts, np.ones(sh, np.float32)])
        rng = trow // rs
        li = (trow - rng * rs).astype(np.int64)
        hv = tq // hsz
        for r in range(nr):
            for h in range(hh):
                mm = (rng == r) & (hv == h)
                tql = tq[mm] - h * hsz          # local target idx in half
                cnt = np.bincount(tql, minlength=hsz)
                order = np.argsort(-cnt, kind="stable")
                cpos = np.empty(hsz, dtype=np.int64)
                cpos[order] = np.arange(hsz)
                nnz = int((cnt > 0).sum())
                ncb[r][h] = max(ncb[r][h], max(1, -(-nnz // 128)))
                per[c][r][h] = dict(tql=tql, li=li[mm], w=wts[mm],
                                    cnt=cnt, cpos=cpos)

    # uniform per-(r,h,cb) column counts across cores ------------------------
    S = [[np.zeros(int(ncb[r][h]), dtype=np.int64) for h in range(hh)]
         for r in range(nr)]
    for c in range(p):
        for r in range(nr):
            for h in range(hh):
                pc = per[c][r][h]
                csort = pc["cnt"][np.argsort(-pc["cnt"], kind="stable")]
                for cb in range(int(ncb[r][h])):
                    blk = csort[cb * 128:(cb + 1) * 128]
                    if len(blk):
                        S[r][h][cb] = max(S[r][h][cb], int(blk.max()))
    colofs = [[np.concatenate([[0], np.cumsum(S[r][h])]) for h in range(hh)]
              for r in range(nr)]
    # global column offsets, order (h, r) major for the device loop
    gofs = {}
    g = 0
    for h in range(hh):
        for r in range(nr):
            gofs[(r, h)] = g
            g += int(S[r][h].sum())
    gcols = max(1, g)

    # block-layout target grid for deg (non-self edges) ----------------------
    TS = np.zeros(nb, dtype=np.int64)
    tg_cnt = []
    for c in range(p):
        m = tshard == c
        cnt = np.bincount(col[m] - c * sh, minlength=npad)
        tg_cnt.append(cnt)
        TS = np.maximum(TS, cnt.reshape(nb, 128).max(axis=1))
    tofs = np.concatenate([[0], np.cumsum(TS)])
    tcols = max(1, int(TS.sum()))

    in_maps = []
    wcat = np.concatenate([np.asarray(Wmu, np.float32),
                           np.asarray(Wls, np.float32)], axis=1)
    bcat = np.concatenate([np.asarray(bmu, np.float32),
                           np.asarray(bls, np.float32)])
    ident = np.eye(128, dtype=np.float32)

    for c in range(p):
        ew_grid = np.zeros((128, gcols), dtype=np.float32)
        gidx = np.zeros(gcols * 128, dtype=np.int64)
        midx = np.zeros(nr * nb * 128, dtype=np.int64)
        for r in range(nr):
            for h in range(hh):
                pc = per[c][r][h]
                cq = pc["cpos"][pc["tql"]]
                o = np.argsort(cq, kind="stable")
                cq_s, li_s, w_s = cq[o], pc["li"][o], pc["w"][o]
                slot = _slot_ranks(cq_s)
                cb = cq_s // 128
                part = cq_s % 128
                gcol = gofs[(r, h)] + colofs[r][h][cb] + slot
                ew_grid[part, gcol] = w_s
                gidx[gcol * 128 + part] = li_s
                # merge index: final row (b,p) -> compact row or zero row
                zrow = int(ncb[r][h]) * 128
                qq = np.arange(hsz)
                mrow = np.where(pc["cnt"] > 0, pc["cpos"], zrow)
                fin = h * hsz + qq
                midx[r * npad + fin] = mrow

        # per-final-block tgrid of non-self edge weights (for deg)
        m = tshard == c
        tq = col[m] - c * sh
        wts = ew[m]
        o = np.argsort(tq, kind="stable")
        tq_s, w_s = tq[o], wts[o]
        slot = _slot_ranks(tq_s)
        tgrid = np.zeros((128, tcols), dtype=np.float32)
        tgrid[tq_s % 128, tofs[tq_s // 128] + slot] = w_s

        in_maps.append({
            "x_tab": x_tab,
            "ew_grid": ew_grid,
            "tgrid": tgrid,
            "gidx": _wrap16(gidx),
            "midx": _wrap16(midx),
            "w1": np.asarray(W1, np.float32),
            "wcat": wcat,
            "b1row": np.asarray(b1, np.float32).reshape(1, -1),
            "bcatrow": bcat.reshape(1, -1),
            "ident": ident,
        })

    meta = dict(ncb=[[int(ncb[r][h]) for h in range(hh)] for r in range(nr)],
                S=[[list(map(int, S[r][h])) for h in range(hh)]
                   for r in range(nr)],
                TS=list(map(int, TS)), gcols=gcols, tcols=tcols,
                gofs={f"{r}_{h}": gofs[(r, h)] for r in range(nr)
                      for h in range(hh)})
    return in_maps, meta


# ----------------------------------------------------------------------------
# device program
# ----------------------------------------------------------------------------

def _build(cfg, meta, split=True):
    import concourse.bacc as bacc
    import concourse.mybir as mybir
    from concourse.tile import TileContext

    p, nb, npad, tbl = cfg["p"], cfg["nb"], cfg["npad"], cfg["tbl"]
    nr, rs, hh, hblk = cfg["nr"], cfg["rs"], cfg["hh"], cfg["hblk"]
    CPC, MBB, SG = cfg["cpc"], cfg["mbb"], cfg["sgrp"]
    ncb, S, TS = meta["ncb"], meta["S"], meta["TS"]
    gcols, tcols = meta["gcols"], meta["tcols"]
    gofs = {tuple(map(int, k.split("_"))): v for k, v in meta["gofs"].items()}
    f32, bf16, i16 = mybir.dt.float32, mybir.dt.bfloat16, mybir.dt.int16
    AX = mybir.AxisListType.X
    OP = mybir.AluOpType
    ACTF = mybir.ActivationFunctionType
    tpr = rs // 128                         # table blocks per range (196)

    nc = bacc.Bacc(num_devices=p)
    x_tab = nc.declare_dram_parameter("x_tab", [tbl, 128], f32, isOutput=False)
    ew_grid = nc.declare_dram_parameter("ew_grid", [128, gcols], f32, isOutput=False)
    tgrid = nc.declare_dram_parameter("tgrid", [128, tcols], f32, isOutput=False)
    gidx = nc.declare_dram_parameter("gidx", [128, gcols * 8], i16, isOutput=False)
    midx = nc.declare_dram_parameter("midx", [128, nr * nb * 8], i16, isOutput=False)
    w1 = nc.declare_dram_parameter("w1", [128, 128], f32, isOutput=False)
    wcat = nc.declare_dram_parameter("wcat", [128, 128], f32, isOutput=False)
    b1row = nc.declare_dram_parameter("b1row", [1, 128], f32, isOutput=False)
    bcatrow = nc.declare_dram_parameter("bcatrow", [1, 128], f32, isOutput=False)
    ident = nc.declare_dram_parameter("ident", [128, 128], f32, isOutput=False)
    out_ext = nc.declare_dram_parameter("out", [npad, 128], f32, isOutput=True)

    with TileContext(nc) as tc:
        with tc.tile_pool(name="dram", bufs=1, space="DRAM") as dram, \
             tc.tile_pool(name="persist", bufs=1) as pp, \
             tc.tile_pool(name="scl", bufs=2) as sclp, \
             tc.tile_pool(name="grid", bufs=cfg["grid_bufs"]) as gp, \
             tc.tile_pool(name="acc", bufs=cfg["acc_bufs"]) as accp, \
             tc.tile_pool(name="mg", bufs=cfg["mg_bufs"]) as mgp, \
             tc.tile_pool(name="work", bufs=cfg["wp_bufs"]) as wp, \
             tc.tile_pool(name="psum", bufs=4, space="PSUM") as psp:

            xs_r = [dram.tile([rs, 128], bf16, tag=f"xs{r}", name=f"xs{r}")
                    for r in range(nr)]
            hs_shard = dram.tile([npad, 128], bf16, tag="hss", name="hs_shard")
            hs_tab = dram.tile([tbl, 128], bf16, tag="hst", name="hs_tab")
            dv_col = dram.tile([npad, 1], f32, tag="dvc", name="dv_col")
            dv_gat = dram.tile([tbl, 1], f32, tag="dvg", name="dv_gat")
            parts = {}
            for r in range(nr):
                for h in range(hh):
                    parts[(r, h)] = dram.tile(
                        [(ncb[r][h] + 1) * 128, 128], f32,
                        tag=f"pt{r}{h}", name=f"part{r}{h}")

            ewg_t = pp.tile([128, gcols], f32, tag="ewg", name="ewg_t")
            gidx_t = pp.tile([128, gcols * 8], i16, tag="gix", name="gidx_t")
            midx_t = pp.tile([128, nr * nb * 8], i16, tag="mix", name="midx_t")
            tg_t = pp.tile([128, tcols], f32, tag="tg", name="tg_t")
            w1_t = pp.tile([128, 128], f32, tag="w1", name="w1_t")
            wcat_t = pp.tile([128, 128], f32, tag="wc", name="wcat_t")
            b1_t = pp.tile([1, 128], f32, tag="b1", name="b1_t")
            bcat_t = pp.tile([1, 128], f32, tag="bc", name="bcat_t")
            id_t = pp.tile([128, 128], f32, tag="id", name="id_t")
            ones_t = pp.tile([1, 128], f32, tag="on", name="ones_t")
            deg_t = pp.tile([128, nb], f32, tag="dg", name="deg_t")
            d2_t = pp.tile([128, nb], f32, tag="d2", name="d2_t")
            dv_t = pp.tile([128, nb], f32, tag="dv", name="dv_t")
            dvf_t = pp.tile([128, p * nb], f32, tag="dvf", name="dvf_t")
            dvrow_t = pp.tile([128, 128], f32, tag="dvr", name="dvrow_t")
            dvcat_t = pp.tile([1, nb * 128], f32, tag="dvx", name="dvcat_t")
            z_t = pp.tile([128, 128], f32, tag="z", name="z_t")

            for t, src in [(ewg_t, ew_grid), (gidx_t, gidx), (midx_t, midx),
                           (tg_t, tgrid), (w1_t, w1), (wcat_t, wcat),
                           (b1_t, b1row), (bcat_t, bcatrow), (id_t, ident)]:
                nc.sync.dma_start(out=t[:], in_=src[:])
            nc.vector.memset(ones_t[:], 1.0)
            nc.vector.memset(z_t[:], 0.0)
            for r in range(nr):
                for h in range(hh):
                    nc.sync.dma_start(
                        out=parts[(r, h)][ncb[r][h] * 128:(ncb[r][h] + 1) * 128, :],
                        in_=z_t[:])

            # deg -> d2 (=1/deg) and dinv; own shard, block layout ----------
            tof = 0
            for b in range(nb):
                if TS[b] > 0:
                    nc.vector.tensor_reduce(deg_t[:, b:b + 1],
                                            tg_t[:, tof:tof + TS[b]],
                                            axis=AX, op=OP.add)
                else:
                    nc.vector.memset(deg_t[:, b:b + 1], 0.0)
                tof += TS[b]
            nc.vector.tensor_scalar_add(deg_t[:], deg_t[:], 1.0)
            nc.vector.reciprocal(d2_t[:], deg_t[:])
            nc.scalar.sqrt(dv_t[:], d2_t[:])
            # dinv rows at partition 0 (bias outer-product lhsT)
            dvr_ps = psp.tile([128, 128], f32, tag="ps", name="dvr_ps")
            nc.tensor.transpose(dvr_ps[:nb, :], dv_t[:, :nb], id_t[:])
            nc.scalar.activation(dvrow_t[:nb, :], dvr_ps[:nb, :], ACTF.Copy)
            nc.sync.dma_start(
                out=dvcat_t[:].rearrange("p (b c) -> p b c", c=128)[0:1, :nb, :],
                in_=dvrow_t[:nb, :])
            # dinv column -> AllGather -> full dinv in block layout
            nc.sync.dma_start(
                out=dv_col[:].rearrange("(b p) one -> p (b one)", p=128),
                in_=dv_t[:])

            groups = [list(range(p))]
            tc.strict_bb_all_engine_barrier()
            nc.gpsimd.collective_compute(
                "AllGather", OP.bypass, replica_groups=groups,
                ins=[dv_col.opt()], outs=[dv_gat.opt()])
            tc.strict_bb_all_engine_barrier()
            nc.sync.dma_start(
                out=dvf_t[:],
                in_=dv_gat[:].rearrange("(g p) one -> p (g one)", p=128))

            # scale pass: xs_r = dinv * x_tab (bf16), per range -------------
            for r in range(nr):
                for g0 in range(0, tpr, SG):
                    ng = min(SG, tpr - g0)
                    xt = sclp.tile([128, SG * 128], f32, tag="sx", name="sx")
                    xo = sclp.tile([128, SG * 128], bf16, tag="so", name="so")
                    base = r * rs + g0 * 128
                    nc.sync.dma_start(
                        out=xt[:, :ng * 128].rearrange("p (g c) -> p g c", c=128),
                        in_=x_tab[base:base + ng * 128, :]
                        .rearrange("(g p) c -> p g c", p=128))
                    for k in range(ng):
                        gb = r * tpr + g0 + k
                        nc.vector.tensor_scalar_mul(
                            xo[:, k * 128:(k + 1) * 128],
                            xt[:, k * 128:(k + 1) * 128],
                            dvf_t[:, gb:gb + 1])
                    nc.sync.dma_start(
                        out=xs_r[r][g0 * 128:(g0 + ng) * 128, :]
                        .rearrange("(g p) c -> p g c", p=128),
                        in_=xo[:, :ng * 128].rearrange("p (g c) -> p g c", c=128))

            # column -> (compact block, last?) map per (r,h)
            cmap = {}
            for r in range(nr):
                for h in range(hh):
                    m = []
                    for cb in range(ncb[r][h]):
                        m += [cb] * S[r][h][cb]
                    cmap[(r, h)] = m

            def aggregate(table_ap, r, h):
                """gather+accumulate columns of (r,h); write compact partials."""
                colmap = cmap[(r, h)]
                total = len(colmap)
                part = parts[(r, h)]
                done = 0
                acc = None
                acc_cb0 = 0
                ACB = 4                       # compact blocks per acc tile
                while done < total:
                    ncall = min(CPC, total - done)
                    grid = gp.tile([128, CPC * 128], bf16, tag="grid",
                                   name="grid")
                    go = gofs[(r, h)] + done
                    nc.gpsimd.dma_gather(
                        out_ap=grid[:, :ncall * 128].rearrange(
                            "p (g c) -> p g c", c=128),
                        in_ap=table_ap,
                        idxs_ap=gidx_t[:, go * 8:(go + ncall) * 8],
                        num_idxs=ncall * 128, num_idxs_reg=ncall * 128,
                        elem_size=128, single_packet=False)
                    for j in range(ncall):
                        lcol = done + j
                        cb = colmap[lcol]
                        if acc is None or cb >= acc_cb0 + ACB:
                            if acc is not None:
                                hi = min(acc_cb0 + ACB, ncb[r][h])
                                nc.sync.dma_start(
                                    out=part[acc_cb0 * 128:hi * 128, :]
                                    .rearrange("(g p) c -> p g c", p=128),
                                    in_=acc[:, :(hi - acc_cb0) * 128]
                                    .rearrange("p (g c) -> p g c", c=128))
                            acc = accp.tile([128, 4 * 128], f32, tag="acc",
                                            name="acc")
                            acc_cb0 = (cb // ACB) * ACB
                        sl = acc[:, (cb - acc_cb0) * 128:(cb - acc_cb0 + 1) * 128]
                        src = grid[:, j * 128:(j + 1) * 128]
                        sc = ewg_t[:, gofs[(r, h)] + lcol:gofs[(r, h)] + lcol + 1]
                        first = (lcol == 0 or colmap[lcol - 1] != cb)
                        if first:
                            nc.vector.tensor_scalar_mul(sl, src, sc)
                        else:
                            nc.vector.scalar_tensor_tensor(
                                sl, src, sc, sl, OP.mult, OP.add)
                    done += ncall
                if acc is not None:
                    hi = ncb[r][h]
                    nc.sync.dma_start(
                        out=part[acc_cb0 * 128:hi * 128, :]
                        .rearrange("(g p) c -> p g c", p=128),
                        in_=acc[:, :(hi - acc_cb0) * 128]
                        .rearrange("p (g c) -> p g c", c=128))

            def merge_epilogue(h, front, wmat, bias_lhsT, bias_rhs, actf,
                               res_dt, res_writer):
                b_lo = h * hblk
                for b0 in range(b_lo, b_lo + hblk, MBB):
                    nbb = min(MBB, b_lo + hblk - b0)
                    M = mgp.tile([128, nr * MBB * 128], f32, tag="mg", name="M")
                    for r in range(nr):
                        s0 = (r * nb + b0) * 128
                        nc.gpsimd.dma_gather(
                            out_ap=M[:, r * MBB * 128:(r * MBB + nbb) * 128]
                            .rearrange("p (g c) -> p g c", c=128),
                            in_ap=parts[(r, h)][:],
                            idxs_ap=midx_t[:, s0 // 16:(s0 + nbb * 128) // 16],
                            num_idxs=nbb * 128, num_idxs_reg=nbb * 128,
                            elem_size=128, single_packet=False)
                    for bi in range(nbb):
                        b = b0 + bi
                        def mg(r):
                            return M[:, (r * MBB + bi) * 128:
                                     (r * MBB + bi + 1) * 128]
                        s1 = wp.tile([128, 128], f32, tag="s1", name="s1")
                        s2 = wp.tile([128, 128], f32, tag="s2", name="s2")
                        nc.vector.tensor_tensor(s1[:], mg(0), mg(1), OP.add)
                        nc.vector.tensor_tensor(s2[:], mg(2), mg(3), OP.add)
                        agg = wp.tile([128, 128], f32, tag="agg", name="agg")
                        nc.vector.tensor_tensor(agg[:], s1[:], s2[:], OP.add)
                        asc = wp.tile([128, 128], f32, tag="asc", name="asc")
                        nc.scalar.activation(asc[:], agg[:], ACTF.Copy,
                                             scale=front[:, b:b + 1])
                        tps = psp.tile([128, 128], f32, tag="ps", name="tps")
                        nc.tensor.transpose(tps[:], asc[:], id_t[:])
                        aggT = wp.tile([128, 128], f32, tag="aggT", name="aggT")
                        nc.scalar.activation(aggT[:], tps[:], ACTF.Copy)
                        zps = psp.tile([128, 128], f32, tag="zps", name="zps")
                        nc.tensor.matmul(zps[:], bias_lhsT(b), bias_rhs[:],
                                         start=True, stop=False)
                        nc.tensor.matmul(zps[:], aggT[:], wmat[:],
                                         start=False, stop=True)
                        res = wp.tile([128, 128], res_dt, tag=f"res{res_dt}",
                                      name="res")
                        nc.scalar.activation(res[:], zps[:], actf)
                        res_writer(b, res)

            # ---- layer 1 ----
            def l1_write(b, res):
                nc.sync.dma_start(out=hs_shard[b * 128:(b + 1) * 128, :],
                                  in_=res[:])

            for h in range(hh):
                for r in range(nr):
                    aggregate(xs_r[r][:], r, h)
                merge_epilogue(
                    h, d2_t, w1_t,
                    lambda b: dvcat_t[:].rearrange(
                        "p (b c) -> p b c", c=128)[0:1, b, :],
                    b1_t, ACTF.Relu, bf16, l1_write)

            # ---- AllGather hs ----
            tc.strict_bb_all_engine_barrier()
            nc.gpsimd.collective_compute(
                "AllGather", OP.bypass, replica_groups=groups,
                ins=[hs_shard.opt()], outs=[hs_tab.opt()])
            tc.strict_bb_all_engine_barrier()

            # ---- layers 2+3 ----
            def l2_write(b, res):
                nc.sync.dma_start(out=out_ext[b * 128:(b + 1) * 128, :],
                                  in_=res[:])

            for h in range(hh):
                for r in range(nr):
                    aggregate(hs_tab[r * rs:(r + 1) * rs, :], r, h)
                merge_epilogue(h, dv_t, wcat_t, lambda b: ones_t[:],
                               bcat_t, ACTF.Copy, f32, l2_write)

    nc.finalize()
    if split:
        _split_excess_waits(nc)
    return nc


# ----------------------------------------------------------------------------
# top-level entry
# ----------------------------------------------------------------------------

_CACHE = {}


def get_built(cfg, meta, split=True):
    key = repr((sorted(cfg.items()), repr(meta), split))
    if key not in _CACHE:
        _CACHE[key] = _build(cfg, meta, split=split)
    return _CACHE[key]


def run(inputs, cfg):
    from concourse.bass_utils import run_bass_kernel_spmd
    in_maps, meta = _preprocess(cfg, **inputs)
    nc = get_built(cfg, meta)
    res = run_bass_kernel_spmd(nc, in_maps, list(range(cfg["p"])))
    return postprocess(res.results, cfg)


def postprocess(results, cfg):
    n, sh, o, p = cfg["n"], cfg["sh"], cfg["o"], cfg["p"]
    mu = np.empty((n, o), dtype=np.float32)
    ls = np.empty((n, o), dtype=np.float32)
    for c in range(p):
        out = results[c]["out"]
        mu[c * sh:(c + 1) * sh] = out[:sh, :o]
        ls[c * sh:(c + 1) * sh] = out[:sh, o:2 * o]
    return mu, ls


def kernel(x, edge_index, edge_attr, W1, b1, Wmu, bmu, Wls, bls):
    return run(dict(x=x, edge_index=edge_index, edge_attr=edge_attr, W1=W1,
                    b1=b1, Wmu=Wmu, bmu=bmu, Wls=Wls, bls=bls), CFG)


# revision 12
# speedup vs baseline: 1.6046x; 1.2248x over previous
"""Self-contained Trainium2 (Bass) kernel for a 3-conv GCN encoder.

reference math (PyG GCNConv with edge weights, symmetric norm, self loops):
    deg[t]  = 1 + sum_{e: col[e]=t} ew[e]
    dinv    = deg ** -0.5 ; d2 = 1/deg
    aggr(X)[t] = dinv[t]*X[t] + sum_{e->t} ew[e] * Xs[src]   (Xs = dinv*X)
    hs = relu(d2 * aggr_raw @ W1 + dinv x b1)      (= dinv * h)
    [mu|ls] = dinv * aggr2_raw @ [Wmu|Wls] + [bmu|bls]

Distribution: nodes target-sharded across 8 cores, identity table order.
Every core receives the full x (bf16) and builds the scaled gather table
locally; only a [npad,1] dinv column is AllGather'd up front, and hs once
in bf16. Edges are laid out in degree-sorted per-(range, half) compact slot
grids for int16 dma_gather (grids exclude self loops and are shared by both
aggregation passes); per-range partial sums are merged per final block with
a 4-way gather + adds (absent entries hit a dedicated zero row) plus the
self-loop row loaded straight from xs_own / hs_shard.
"""

import numpy as np


CFG = dict(n=100000, e=1600000, p=8, f=128, h=128, o=64,
           sh=12500, nb=98, npad=12544, tbl=100352, nr=4, rs=25088,
           hh=2, hblk=49, cpc=64, mbb=4, acb=4, sgrp=14, sclg=14,
           grid_bufs=3, acc_bufs=8, mg_bufs=4, wp_bufs=4, barriers=True)


# ----------------------------------------------------------------------------
# walrus compat shim: this env's walrus rejects >1 sync-wait per instruction
# (and any wait on InstDrain); hoist excess waits onto InstEventSemaphore.
# ----------------------------------------------------------------------------

def _split_excess_waits(nc, max_inline=1):
    import concourse.mybir as mybir
    n_moved = 0
    for fn in nc.m.functions:
        for bb in fn.blocks:
            new_insts = []
            for inst in bb.instructions:
                si = inst.sync_info
                if si is not None and si.on_wait:
                    keep = 0 if isinstance(inst, mybir.InstDrain) else max_inline
                    if isinstance(inst, mybir.InstEventSemaphore):
                        keep = max(keep, 1)
                    waits = list(si.on_wait)
                    if len(waits) > keep:
                        hoist = waits[:-keep] if keep else waits
                        inline = waits[-keep:] if keep else []
                        for w in hoist:
                            ev = mybir.InstEventSemaphore(
                                name=nc.get_next_instruction_name(), ins=[], outs=[])
                            ev.engine = inst.engine
                            ev.sync_info = mybir.SyncInfo(on_wait=[w], on_update=[])
                            new_insts.append(ev)
                            n_moved += 1
                        si.on_wait = inline
                new_insts.append(inst)
            bb.instructions[:] = new_insts
    return n_moved


# ----------------------------------------------------------------------------
# host preprocessing (index/shuffle/dtype-cast only; FP math stays on device)
# ----------------------------------------------------------------------------

def _wrap16(idxs):
    """int16 index stream -> [128, n/16] tile (16-wrapped, 8x replicated)."""
    n = len(idxs)
    assert n % 16 == 0
    t = np.zeros((128, n // 16), dtype=np.int16)
    blk = idxs.reshape(n // 16, 16).T.astype(np.int16)
    for k in range(8):
        t[16 * k:16 * (k + 1), :] = blk
    return t


def _slot_ranks(sorted_keys):
    """for a sorted int array, rank of each element within its value-group."""
    n = len(sorted_keys)
    if n == 0:
        return np.zeros(0, dtype=np.int64)
    starts = np.r_[0, np.flatnonzero(np.diff(sorted_keys)) + 1]
    group_start = np.repeat(starts, np.diff(np.r_[starts, n]))
    return np.arange(n) - group_start


def _preprocess(cfg, x, edge_index, edge_attr, W1, b1, Wmu, bmu, Wls, bls):
    p, sh, nb, npad = cfg["p"], cfg["sh"], cfg["nb"], cfg["npad"]
    nr, rs, hh, hblk = cfg["nr"], cfg["rs"], cfg["hh"], cfg["hblk"]
    hsz = hblk * 128

    row = np.asarray(edge_index[0], dtype=np.int64)
    col = np.asarray(edge_index[1], dtype=np.int64)
    ew = np.asarray(edge_attr, dtype=np.float32)
    x = np.asarray(x, dtype=np.float32)

    # full x in (identity) table order, padded per shard, bf16 ---------------
    x_tab = np.zeros((cfg["tbl"], 128), dtype=np.float32)
    for c in range(p):
        x_tab[c * npad:c * npad + sh] = x[c * sh:(c + 1) * sh]
    import ml_dtypes
    x_tab = x_tab.astype(ml_dtypes.bfloat16)

    ss = row // sh
    trow_all = ss * npad + (row - ss * sh)
    tshard = col // sh

    per = [[[None] * hh for _ in range(nr)] for _ in range(p)]
    ncb = np.zeros((nr, hh), dtype=np.int64)
    for c in range(p):
        m = tshard == c
        tq = col[m] - c * sh
        trow = trow_all[m]
        wts = ew[m]
        rng = trow // rs
        li = (trow - rng * rs).astype(np.int64)
        hv = tq // hsz
        for r in range(nr):
            for h in range(hh):
                mm = (rng == r) & (hv == h)
                tql = tq[mm] - h * hsz
                cnt = np.bincount(tql, minlength=hsz)
                order = np.argsort(-cnt, kind="stable")
                cpos = np.empty(hsz, dtype=np.int64)
                cpos[order] = np.arange(hsz)
                nnz = int((cnt > 0).sum())
                ncb[r][h] = max(ncb[r][h], max(1, -(-nnz // 128)))
                per[c][r][h] = dict(tql=tql, li=li[mm], w=wts[mm],
                                    cnt=cnt, cpos=cpos)

    # subgroup (CB, j) column schedule: partition p holds compact rows
    # CB*512 + 4p + j, so partial tiles write as 1KB quad descriptors.
    nCB = [[-(-int(ncb[r][h]) // 4) for h in range(hh)] for r in range(nr)]
    S = [[np.zeros(nCB[r][h] * 4, dtype=np.int64) for h in range(hh)]
         for r in range(nr)]
    for c in range(p):
        for r in range(nr):
            for h in range(hh):
                pc = per[c][r][h]
                csort = pc["cnt"][np.argsort(-pc["cnt"], kind="stable")]
                for sg in range(nCB[r][h] * 4):
                    blk = csort[sg * 128:(sg + 1) * 128]
                    if len(blk):
                        S[r][h][sg] = max(S[r][h][sg], int(blk.max()))
    colofs = [[np.concatenate([[0], np.cumsum(S[r][h])]) for h in range(hh)]
              for r in range(nr)]
    gofs = {}
    g = 0
    for h in range(hh):
        for r in range(nr):
            gofs[(r, h)] = g
            g += int(S[r][h].sum())
    gcols = max(1, g)

    # block-layout target grid of non-self edge weights (for deg) ------------
    TS = np.zeros(nb, dtype=np.int64)
    for c in range(p):
        m = tshard == c
        cnt = np.bincount(col[m] - c * sh, minlength=npad)
        TS = np.maximum(TS, cnt.reshape(nb, 128).max(axis=1))
    tofs = np.concatenate([[0], np.cumsum(TS)])
    tcols = max(1, int(TS.sum()))

    in_maps = []
    wcat = np.concatenate([np.asarray(Wmu, np.float32),
                           np.asarray(Wls, np.float32)], axis=1)
    bcat = np.concatenate([np.asarray(bmu, np.float32),
                           np.asarray(bls, np.float32)])
    ident = np.eye(128, dtype=np.float32)

    for c in range(p):
        ew_grid = np.zeros((128, gcols), dtype=np.float32)
        gidx = np.zeros(gcols * 128, dtype=np.int64)
        midx = np.zeros(nr * nb * 128, dtype=np.int64)
        for r in range(nr):
            for h in range(hh):
                pc = per[c][r][h]
                rank = pc["cpos"][pc["tql"]]        # rank in cnt-desc order
                o = np.argsort(rank, kind="stable")
                rank_s, li_s, w_s = rank[o], pc["li"][o], pc["w"][o]
                slot = _slot_ranks(rank_s)
                sg_s = rank_s // 128
                part = rank_s % 128
                gcol = gofs[(r, h)] + colofs[r][h][sg_s] + slot
                ew_grid[part, gcol] = w_s
                gidx[gcol * 128 + part] = li_s
                # DRAM row of rank k: quad-interleaved within each 512-group
                zrow = nCB[r][h] * 512
                qq = np.arange(hsz)
                rk = pc["cpos"]
                quad = (rk // 512) * 512 + (rk % 128) * 4 + (rk // 128) % 4
                mrow = np.where(pc["cnt"] > 0, quad, zrow)
                midx[r * npad + h * hsz + qq] = mrow

        m = tshard == c
        tq = col[m] - c * sh
        wts = ew[m]
        o = np.argsort(tq, kind="stable")
        tq_s, w_s = tq[o], wts[o]
        slot = _slot_ranks(tq_s)
        tgrid = np.zeros((128, tcols), dtype=np.float32)
        tgrid[tq_s % 128, tofs[tq_s // 128] + slot] = w_s

        in_maps.append({
            "x_tab": x_tab,
            "x_own": np.ascontiguousarray(x_tab[c * npad:(c + 1) * npad]),
            "ew_grid": ew_grid,
            "tgrid": tgrid,
            "gidx": _wrap16(gidx),
            "midx": _wrap16(midx),
            "w1": np.asarray(W1, np.float32),
            "wcat": wcat,
            "b1row": np.asarray(b1, np.float32).reshape(1, -1),
            "bcatrow": bcat.reshape(1, -1),
            "ident": ident,
        })

    meta = dict(ncb=[[int(nCB[r][h]) for h in range(hh)] for r in range(nr)],
                S=[[list(map(int, S[r][h])) for h in range(hh)]
                   for r in range(nr)],
                TS=list(map(int, TS)), gcols=gcols, tcols=tcols,
                gofs={f"{r}_{h}": gofs[(r, h)] for r in range(nr)
                      for h in range(hh)})
    return in_maps, meta


# ----------------------------------------------------------------------------
# device program
# ----------------------------------------------------------------------------

def _build(cfg, meta, split=True):
    import concourse.bacc as bacc
    import concourse.mybir as mybir
    from concourse.tile import TileContext

    p, nb, npad, tbl = cfg["p"], cfg["nb"], cfg["npad"], cfg["tbl"]
    nr, rs, hh, hblk = cfg["nr"], cfg["rs"], cfg["hh"], cfg["hblk"]
    CPC, MBB, ACB, SG = cfg["cpc"], cfg["mbb"], cfg["acb"], cfg["sgrp"]
    SCLG = cfg["sclg"]
    ncb, S, TS = meta["ncb"], meta["S"], meta["TS"]
    gcols, tcols = meta["gcols"], meta["tcols"]
    gofs = {tuple(map(int, k.split("_"))): v for k, v in meta["gofs"].items()}
    f32, bf16, i16 = mybir.dt.float32, mybir.dt.bfloat16, mybir.dt.int16
    AX = mybir.AxisListType.X
    OP = mybir.AluOpType
    ACTF = mybir.ActivationFunctionType
    tgrp = rs // 512                        # 512-row groups per range (49)

    nc = bacc.Bacc(num_devices=p)
    x_tab = nc.declare_dram_parameter("x_tab", [tbl, 128], bf16, isOutput=False)
    x_own = nc.declare_dram_parameter("x_own", [npad, 128], bf16, isOutput=False)
    ew_grid = nc.declare_dram_parameter("ew_grid", [128, gcols], f32, isOutput=False)
    tgrid = nc.declare_dram_parameter("tgrid", [128, tcols], f32, isOutput=False)
    gidx = nc.declare_dram_parameter("gidx", [128, gcols * 8], i16, isOutput=False)
    midx = nc.declare_dram_parameter("midx", [128, nr * nb * 8], i16, isOutput=False)
    w1 = nc.declare_dram_parameter("w1", [128, 128], f32, isOutput=False)
    wcat = nc.declare_dram_parameter("wcat", [128, 128], f32, isOutput=False)
    b1row = nc.declare_dram_parameter("b1row", [1, 128], f32, isOutput=False)
    bcatrow = nc.declare_dram_parameter("bcatrow", [1, 128], f32, isOutput=False)
    ident = nc.declare_dram_parameter("ident", [128, 128], f32, isOutput=False)
    out_ext = nc.declare_dram_parameter("out", [npad, 128], f32, isOutput=True)

    with TileContext(nc) as tc:
        with tc.tile_pool(name="dram", bufs=1, space="DRAM") as dram, \
             tc.tile_pool(name="persist", bufs=1) as pp, \
             tc.tile_pool(name="scl", bufs=2) as sclp, \
             tc.tile_pool(name="grid", bufs=cfg["grid_bufs"]) as gp, \
             tc.tile_pool(name="acc", bufs=cfg["acc_bufs"]) as accp, \
             tc.tile_pool(name="mg", bufs=cfg["mg_bufs"]) as mgp, \
             tc.tile_pool(name="work", bufs=cfg["wp_bufs"]) as wp, \
             tc.tile_pool(name="psum", bufs=4, space="PSUM") as psp:

            xs_r = [dram.tile([rs, 128], bf16, tag=f"xs{r}", name=f"xs{r}")
                    for r in range(nr)]
            xs_own = dram.tile([npad, 128], bf16, tag="xso", name="xs_own")
            hs_shard = dram.tile([npad, 128], bf16, tag="hss", name="hs_shard")
            hs_tab = dram.tile([tbl, 128], bf16, tag="hst", name="hs_tab")
            dv_col = dram.tile([npad, 1], f32, tag="dvc", name="dv_col")
            dv_gat = dram.tile([tbl, 1], f32, tag="dvg", name="dv_gat")
            parts = {}
            for r in range(nr):
                for h in range(hh):
                    parts[(r, h)] = dram.tile(
                        [ncb[r][h] * 512 + 128, 128], bf16,
                        tag=f"pt{r}{h}", name=f"part{r}{h}")

            ewg_t = pp.tile([128, gcols], f32, tag="ewg", name="ewg_t")
            gidx_t = pp.tile([128, gcols * 8], i16, tag="gix", name="gidx_t")
            midx_t = pp.tile([128, nr * nb * 8], i16, tag="mix", name="midx_t")
            tg_t = pp.tile([128, tcols], f32, tag="tg", name="tg_t")
            w1_t = pp.tile([128, 128], f32, tag="w1", name="w1_t")
            wcat_t = pp.tile([128, 128], f32, tag="wc", name="wcat_t")
            b1_t = pp.tile([1, 128], f32, tag="b1", name="b1_t")
            bcat_t = pp.tile([1, 128], f32, tag="bc", name="bcat_t")
            id_t = pp.tile([128, 128], f32, tag="id", name="id_t")
            ones_t = pp.tile([1, 128], f32, tag="on", name="ones_t")
            deg_t = pp.tile([128, nb], f32, tag="dg", name="deg_t")
            d2_t = pp.tile([128, nb], f32, tag="d2", name="d2_t")
            dv_t = pp.tile([128, nb], f32, tag="dv", name="dv_t")
            dvq_t = pp.tile([128, (tbl // 512) * 4], f32, tag="dvq",
                            name="dvq_t")
            z_t = pp.tile([128, 128], bf16, tag="z", name="z_t")

            for t, src in [(ewg_t, ew_grid), (gidx_t, gidx), (midx_t, midx),
                           (tg_t, tgrid), (w1_t, w1), (wcat_t, wcat),
                           (b1_t, b1row), (bcat_t, bcatrow), (id_t, ident)]:
                nc.sync.dma_start(out=t[:], in_=src[:])
            nc.vector.memset(ones_t[:], 1.0)
            nc.vector.memset(z_t[:], 0.0)
            for r in range(nr):
                for h in range(hh):
                    nc.sync.dma_start(
                        out=parts[(r, h)][ncb[r][h] * 512:
                                          ncb[r][h] * 512 + 128, :],
                        in_=z_t[:])

            # deg -> d2 (=1/deg) and dinv; own shard, block layout ----------
            tof = 0
            for b in range(nb):
                if TS[b] > 0:
                    nc.vector.tensor_reduce(deg_t[:, b:b + 1],
                                            tg_t[:, tof:tof + TS[b]],
                                            axis=AX, op=OP.add)
                else:
                    nc.vector.memset(deg_t[:, b:b + 1], 0.0)
                tof += TS[b]
            nc.vector.tensor_scalar_add(deg_t[:], deg_t[:], 1.0)
            nc.vector.reciprocal(d2_t[:], deg_t[:])
            nc.scalar.sqrt(dv_t[:], d2_t[:])
            nc.sync.dma_start(
                out=dv_col[:].rearrange("(b p) one -> p (b one)", p=128),
                in_=dv_t[:])

            groups = [list(range(p))]
            tc.strict_bb_all_engine_barrier()
            nc.gpsimd.collective_compute(
                "AllGather", OP.bypass, replica_groups=groups,
                ins=[dv_col.opt()], outs=[dv_gat.opt()])
            tc.strict_bb_all_engine_barrier()
            nc.sync.dma_start(
                out=dvq_t[:].rearrange("p (g j) -> p g j", j=4),
                in_=dv_gat[:].rearrange("(g p j) one -> p g (j one)",
                                        p=128, j=4))

            # xs_own = dinv * x_own (overlaps the start of the scale pass)
            for b0 in range(0, nb, SG):
                nbb = min(SG, nb - b0)
                xt = sclp.tile([128, SG * 128], bf16, tag="sxo", name="sxo")
                nc.sync.dma_start(
                    out=xt[:, :nbb * 128].rearrange("p (g c) -> p g c", c=128),
                    in_=x_own[b0 * 128:(b0 + nbb) * 128, :]
                    .rearrange("(g p) c -> p g c", p=128))
                for k in range(nbb):
                    nc.vector.tensor_scalar_mul(
                        xt[:, k * 128:(k + 1) * 128],
                        xt[:, k * 128:(k + 1) * 128],
                        dv_t[:, b0 + k:b0 + k + 1])
                nc.sync.dma_start(
                    out=xs_own[b0 * 128:(b0 + nbb) * 128, :]
                    .rearrange("(g p) c -> p g c", p=128),
                    in_=xt[:, :nbb * 128].rearrange("p (g c) -> p g c", c=128))

            # scale pass: xs_r = dinv * x_tab (bf16, 1KB descriptors) --------
            for r in range(nr):
                for g0 in range(0, tgrp, SCLG):
                    ngg = min(SCLG, tgrp - g0)
                    xt = sclp.tile([128, SCLG * 512], bf16, tag="sx", name="sx")
                    base = r * rs + g0 * 512
                    nc.sync.dma_start(
                        out=xt[:, :ngg * 512].rearrange(
                            "p (g jc) -> p g jc", jc=512),
                        in_=x_tab[base:base + ngg * 512, :]
                        .rearrange("(g p j) c -> p g (j c)", p=128, j=4))
                    for k in range(ngg):
                        gq = (r * tgrp + g0 + k) * 4
                        for j in range(4):
                            sl = xt[:, k * 512 + j * 128:k * 512 + (j + 1) * 128]
                            nc.vector.tensor_scalar_mul(
                                sl, sl, dvq_t[:, gq + j:gq + j + 1])
                    nc.sync.dma_start(
                        out=xs_r[r][g0 * 512:(g0 + ngg) * 512, :]
                        .rearrange("(g p j) c -> p g (j c)", p=128, j=4),
                        in_=xt[:, :ngg * 512].rearrange(
                            "p (g jc) -> p g jc", jc=512))

            # column -> compact block map per (r,h)
            cmap = {}
            for r in range(nr):
                for h in range(hh):
                    m = []
                    for sg in range(ncb[r][h] * 4):
                        m += [sg] * S[r][h][sg]
                    cmap[(r, h)] = m

            def aggregate(table_ap, r, h):
                colmap = cmap[(r, h)]
                Svec = S[r][h]
                total = len(colmap)
                part = parts[(r, h)]

                def flush(accb, CB):
                    nc.sync.dma_start(
                        out=part[CB * 512:(CB + 1) * 512, :]
                        .rearrange("(p j) c -> p (j c)", p=128),
                        in_=accb[:])

                done = 0
                acc = accb = None
                cur_cb = -1
                while done < total:
                    ncall = min(CPC, total - done)
                    grid = gp.tile([128, CPC * 128], bf16, tag="grid",
                                   name="grid")
                    go = gofs[(r, h)] + done
                    nc.gpsimd.dma_gather(
                        out_ap=grid[:, :ncall * 128].rearrange(
                            "p (g c) -> p g c", c=128),
                        in_ap=table_ap,
                        idxs_ap=gidx_t[:, go * 8:(go + ncall) * 8],
                        num_idxs=ncall * 128, num_idxs_reg=ncall * 128,
                        elem_size=128, single_packet=False)
                    for j in range(ncall):
                        lcol = done + j
                        sg = colmap[lcol]
                        CB, sj = sg // 4, sg % 4
                        if CB != cur_cb:
                            if accb is not None:
                                flush(accb, cur_cb)
                            acc = accp.tile([128, 512], f32, tag="acc",
                                            name="acc")
                            accb = accp.tile([128, 512], bf16, tag="accb",
                                             name="accb")
                            cur_cb = CB
                            if any(Svec[CB * 4 + q] == 0 for q in range(4)):
                                nc.vector.memset(accb[:], 0.0)
                        sl = acc[:, sj * 128:(sj + 1) * 128]
                        slb = accb[:, sj * 128:(sj + 1) * 128]
                        src = grid[:, j * 128:(j + 1) * 128]
                        sc = ewg_t[:, gofs[(r, h)] + lcol:gofs[(r, h)] + lcol + 1]
                        first = (lcol == 0 or colmap[lcol - 1] != sg)
                        last = (lcol + 1 == total or colmap[lcol + 1] != sg)
                        if first and last:
                            nc.vector.tensor_scalar_mul(slb, src, sc)
                        elif first:
                            nc.vector.tensor_scalar_mul(sl, src, sc)
                        elif last:
                            nc.vector.scalar_tensor_tensor(
                                slb, src, sc, sl, OP.mult, OP.add)
                        else:
                            nc.vector.scalar_tensor_tensor(
                                sl, src, sc, sl, OP.mult, OP.add)
                    done += ncall
                if accb is not None:
                    flush(accb, cur_cb)

            def merge_epilogue(h, own_src, front, wmat, bias_rhs,
                               actf, res_dt, res_writer, res_scale=None):
                b_lo = h * hblk
                for b0 in range(b_lo, b_lo + hblk, MBB):
                    nbb = min(MBB, b_lo + hblk - b0)
                    M = mgp.tile([128, nr * MBB * 128], bf16, tag="mg", name="M")
                    own = mgp.tile([128, MBB * 128], bf16, tag="own",
                                   name="own")
                    nc.sync.dma_start(
                        out=own[:, :nbb * 128].rearrange(
                            "p (g c) -> p g c", c=128),
                        in_=own_src[b0 * 128:(b0 + nbb) * 128, :]
                        .rearrange("(g p) c -> p g c", p=128))
                    for r in range(nr):
                        s0 = (r * nb + b0) * 128
                        nc.gpsimd.dma_gather(
                            out_ap=M[:, r * MBB * 128:(r * MBB + nbb) * 128]
                            .rearrange("p (g c) -> p g c", c=128),
                            in_ap=parts[(r, h)][:],
                            idxs_ap=midx_t[:, s0 // 16:(s0 + nbb * 128) // 16],
                            num_idxs=nbb * 128, num_idxs_reg=nbb * 128,
                            elem_size=128, single_packet=False)
                    for bi in range(nbb):
                        b = b0 + bi
                        def mg(r):
                            return M[:, (r * MBB + bi) * 128:
                                     (r * MBB + bi + 1) * 128]
                        s1 = wp.tile([128, 128], bf16, tag="s1", name="s1")
                        s2 = wp.tile([128, 128], bf16, tag="s2", name="s2")
                        nc.vector.tensor_tensor(s1[:], mg(0), mg(1), OP.add)
                        nc.vector.tensor_tensor(s2[:], mg(2), mg(3), OP.add)
                        s3 = wp.tile([128, 128], bf16, tag="s3", name="s3")
                        nc.vector.tensor_tensor(s3[:], s1[:], s2[:], OP.add)
                        agg = wp.tile([128, 128], f32, tag="agg", name="agg")
                        nc.vector.tensor_tensor(
                            agg[:], s3[:],
                            own[:, bi * 128:(bi + 1) * 128], OP.add)
                        asc = wp.tile([128, 128], f32, tag="asc", name="asc")
                        nc.scalar.activation(asc[:], agg[:], ACTF.Copy,
                                             scale=front[:, b:b + 1])
                        tps = psp.tile([128, 128], f32, tag="ps", name="tps")
                        nc.tensor.transpose(tps[:], asc[:], id_t[:])
                        aggT = wp.tile([128, 128], f32, tag="aggT", name="aggT")
                        nc.scalar.activation(aggT[:], tps[:], ACTF.Copy)
                        zps = psp.tile([128, 128], f32, tag="zps", name="zps")
                        nc.tensor.matmul(zps[:], ones_t[:], bias_rhs[:],
                                         start=True, stop=False)
                        nc.tensor.matmul(zps[:], aggT[:], wmat[:],
                                         start=False, stop=True)
                        res = wp.tile([128, 128], res_dt, tag=f"res{res_dt}",
                                      name="res")
                        if res_scale is not None:
                            nc.scalar.activation(res[:], zps[:], actf,
                                                 scale=res_scale[:, b:b + 1])
                        else:
                            nc.scalar.activation(res[:], zps[:], actf)
                        res_writer(b, res)

            # ---- layer 1 ----
            def l1_write(b, res):
                nc.sync.dma_start(out=hs_shard[b * 128:(b + 1) * 128, :],
                                  in_=res[:])

            for h in range(hh):
                for r in range(nr):
                    aggregate(xs_r[r][:], r, h)
                merge_epilogue(h, xs_own, dv_t, w1_t, b1_t,
                               ACTF.Relu, bf16, l1_write, res_scale=dv_t)

            # ---- AllGather hs ----
            if cfg["barriers"]:
                tc.strict_bb_all_engine_barrier()
            nc.gpsimd.collective_compute(
                "AllGather", OP.bypass, replica_groups=groups,
                ins=[hs_shard.opt()], outs=[hs_tab.opt()])
            if cfg["barriers"]:
                tc.strict_bb_all_engine_barrier()

            # ---- layers 2+3 ----
            def l2_write(b, res):
                nc.sync.dma_start(out=out_ext[b * 128:(b + 1) * 128, :],
                                  in_=res[:])

            for h in range(hh):
                for r in range(nr):
                    aggregate(hs_tab[r * rs:(r + 1) * rs, :], r, h)
                merge_epilogue(h, hs_shard, dv_t, wcat_t,
                               bcat_t, ACTF.Copy, f32, l2_write)

    nc.finalize()
    if split:
        _split_excess_waits(nc)
    return nc


# ----------------------------------------------------------------------------
# top-level entry
# ----------------------------------------------------------------------------

_CACHE = {}


def get_built(cfg, meta, split=True):
    key = repr((sorted(cfg.items()), repr(meta), split))
    if key not in _CACHE:
        _CACHE[key] = _build(cfg, meta, split=split)
    return _CACHE[key]


def run(inputs, cfg):
    from concourse.bass_utils import run_bass_kernel_spmd
    in_maps, meta = _preprocess(cfg, **inputs)
    nc = get_built(cfg, meta)
    res = run_bass_kernel_spmd(nc, in_maps, list(range(cfg["p"])))
    return postprocess(res.results, cfg)


def postprocess(results, cfg):
    n, sh, o, p = cfg["n"], cfg["sh"], cfg["o"], cfg["p"]
    mu = np.empty((n, o), dtype=np.float32)
    ls = np.empty((n, o), dtype=np.float32)
    for c in range(p):
        out = results[c]["out"]
        mu[c * sh:(c + 1) * sh] = out[:sh, :o]
        ls[c * sh:(c + 1) * sh] = out[:sh, o:2 * o]
    return mu, ls


def kernel(x, edge_index, edge_attr, W1, b1, Wmu, bmu, Wls, bls):
    return run(dict(x=x, edge_index=edge_index, edge_attr=edge_attr, W1=W1,
                    b1=b1, Wmu=Wmu, bmu=bmu, Wls=Wls, bls=bls), CFG)


# revision 13
# speedup vs baseline: 1.6196x; 1.0093x over previous
"""Self-contained Trainium2 (Bass) kernel for a 3-conv GCN encoder.

reference math (PyG GCNConv with edge weights, symmetric norm, self loops):
    deg[t]  = 1 + sum_{e: col[e]=t} ew[e]
    dinv    = deg ** -0.5 ; d2 = 1/deg
    aggr(X)[t] = dinv[t]*X[t] + sum_{e->t} ew[e] * Xs[src]   (Xs = dinv*X)
    hs = relu(d2 * aggr_raw @ W1 + dinv x b1)      (= dinv * h)
    [mu|ls] = dinv * aggr2_raw @ [Wmu|Wls] + [bmu|bls]

Distribution: nodes target-sharded across 8 cores, identity table order.
Every core receives the full x (bf16) and builds the scaled gather table
locally; only a [npad,1] dinv column is AllGather'd up front, and hs once
in bf16. Edges are laid out in degree-sorted per-(range, half) compact slot
grids for int16 dma_gather (grids exclude self loops and are shared by both
aggregation passes); per-range partial sums are merged per final block with
a 4-way gather + adds (absent entries hit a dedicated zero row) plus the
self-loop row loaded straight from xs_own / hs_shard.
"""

import numpy as np


CFG = dict(n=100000, e=1600000, p=8, f=128, h=128, o=64,
           sh=12500, nb=98, npad=12544, tbl=100352, nr=4, rs=25088,
           hh=2, hblk=49, cpc=64, mbb=4, acb=4, sgrp=14, sclg=14,
           grid_bufs=3, acc_bufs=8, mg_bufs=4, wp_bufs=4, barriers=True)


# ----------------------------------------------------------------------------
# walrus compat shim: this env's walrus rejects >1 sync-wait per instruction
# (and any wait on InstDrain); hoist excess waits onto InstEventSemaphore.
# ----------------------------------------------------------------------------

def _split_excess_waits(nc, max_inline=1):
    import concourse.mybir as mybir
    n_moved = 0
    for fn in nc.m.functions:
        for bb in fn.blocks:
            new_insts = []
            for inst in bb.instructions:
                si = inst.sync_info
                if si is not None and si.on_wait:
                    keep = 0 if isinstance(inst, mybir.InstDrain) else max_inline
                    if isinstance(inst, mybir.InstEventSemaphore):
                        keep = max(keep, 1)
                    waits = list(si.on_wait)
                    if len(waits) > keep:
                        hoist = waits[:-keep] if keep else waits
                        inline = waits[-keep:] if keep else []
                        for w in hoist:
                            ev = mybir.InstEventSemaphore(
                                name=nc.get_next_instruction_name(), ins=[], outs=[])
                            ev.engine = inst.engine
                            ev.sync_info = mybir.SyncInfo(on_wait=[w], on_update=[])
                            new_insts.append(ev)
                            n_moved += 1
                        si.on_wait = inline
                new_insts.append(inst)
            bb.instructions[:] = new_insts
    return n_moved


# ----------------------------------------------------------------------------
# host preprocessing (index/shuffle/dtype-cast only; FP math stays on device)
# ----------------------------------------------------------------------------

def _wrap16(idxs):
    """int16 index stream -> [128, n/16] tile (16-wrapped, 8x replicated)."""
    n = len(idxs)
    assert n % 16 == 0
    t = np.zeros((128, n // 16), dtype=np.int16)
    blk = idxs.reshape(n // 16, 16).T.astype(np.int16)
    for k in range(8):
        t[16 * k:16 * (k + 1), :] = blk
    return t


def _slot_ranks(sorted_keys):
    """for a sorted int array, rank of each element within its value-group."""
    n = len(sorted_keys)
    if n == 0:
        return np.zeros(0, dtype=np.int64)
    starts = np.r_[0, np.flatnonzero(np.diff(sorted_keys)) + 1]
    group_start = np.repeat(starts, np.diff(np.r_[starts, n]))
    return np.arange(n) - group_start


def _preprocess(cfg, x, edge_index, edge_attr, W1, b1, Wmu, bmu, Wls, bls):
    p, sh, nb, npad = cfg["p"], cfg["sh"], cfg["nb"], cfg["npad"]
    nr, rs, hh, hblk = cfg["nr"], cfg["rs"], cfg["hh"], cfg["hblk"]
    hsz = hblk * 128

    row = np.asarray(edge_index[0], dtype=np.int64)
    col = np.asarray(edge_index[1], dtype=np.int64)
    ew = np.asarray(edge_attr, dtype=np.float32)
    x = np.asarray(x, dtype=np.float32)

    # full x in (identity) table order, padded per shard, bf16 ---------------
    x_tab = np.zeros((cfg["tbl"], 128), dtype=np.float32)
    for c in range(p):
        x_tab[c * npad:c * npad + sh] = x[c * sh:(c + 1) * sh]
    import ml_dtypes
    x_tab = x_tab.astype(ml_dtypes.bfloat16)

    ss = row // sh
    trow_all = ss * npad + (row - ss * sh)
    tshard = col // sh

    per = [[[None] * hh for _ in range(nr)] for _ in range(p)]
    ncb = np.zeros((nr, hh), dtype=np.int64)
    for c in range(p):
        m = tshard == c
        tq = col[m] - c * sh
        trow = trow_all[m]
        wts = ew[m]
        rng = trow // rs
        li = (trow - rng * rs).astype(np.int64)
        hv = tq // hsz
        for r in range(nr):
            for h in range(hh):
                mm = (rng == r) & (hv == h)
                tql = tq[mm] - h * hsz
                cnt = np.bincount(tql, minlength=hsz)
                order = np.argsort(-cnt, kind="stable")
                cpos = np.empty(hsz, dtype=np.int64)
                cpos[order] = np.arange(hsz)
                nnz = int((cnt > 0).sum())
                ncb[r][h] = max(ncb[r][h], max(1, -(-nnz // 128)))
                per[c][r][h] = dict(tql=tql, li=li[mm], w=wts[mm],
                                    cnt=cnt, cpos=cpos)

    # subgroup (CB, j) column schedule: partition p holds compact rows
    # CB*512 + 4p + j, so partial tiles write as 1KB quad descriptors.
    nCB = [[-(-int(ncb[r][h]) // 4) for h in range(hh)] for r in range(nr)]
    S = [[np.zeros(nCB[r][h] * 4, dtype=np.int64) for h in range(hh)]
         for r in range(nr)]
    for c in range(p):
        for r in range(nr):
            for h in range(hh):
                pc = per[c][r][h]
                csort = pc["cnt"][np.argsort(-pc["cnt"], kind="stable")]
                for sg in range(nCB[r][h] * 4):
                    blk = csort[sg * 128:(sg + 1) * 128]
                    if len(blk):
                        S[r][h][sg] = max(S[r][h][sg], int(blk.max()))
    colofs = [[np.concatenate([[0], np.cumsum(S[r][h])]) for h in range(hh)]
              for r in range(nr)]
    gofs = {}
    g = 0
    for h in range(hh):
        for r in range(nr):
            gofs[(r, h)] = g
            g += int(S[r][h].sum())
    gcols = max(1, g)

    # block-layout target grid of non-self edge weights (for deg) ------------
    TS = np.zeros(nb, dtype=np.int64)
    for c in range(p):
        m = tshard == c
        cnt = np.bincount(col[m] - c * sh, minlength=npad)
        TS = np.maximum(TS, cnt.reshape(nb, 128).max(axis=1))
    tofs = np.concatenate([[0], np.cumsum(TS)])
    tcols = max(1, int(TS.sum()))

    in_maps = []
    wcat = np.concatenate([np.asarray(Wmu, np.float32),
                           np.asarray(Wls, np.float32)], axis=1)
    bcat = np.concatenate([np.asarray(bmu, np.float32),
                           np.asarray(bls, np.float32)])
    ident = np.eye(128, dtype=np.float32)

    for c in range(p):
        ew_grid = np.zeros((128, gcols), dtype=np.float32)
        gidx = np.zeros(gcols * 128, dtype=np.int64)
        midx = np.zeros(nr * nb * 128, dtype=np.int64)
        for r in range(nr):
            for h in range(hh):
                pc = per[c][r][h]
                rank = pc["cpos"][pc["tql"]]        # rank in cnt-desc order
                o = np.argsort(rank, kind="stable")
                rank_s, li_s, w_s = rank[o], pc["li"][o], pc["w"][o]
                slot = _slot_ranks(rank_s)
                sg_s = rank_s // 128
                part = rank_s % 128
                gcol = gofs[(r, h)] + colofs[r][h][sg_s] + slot
                ew_grid[part, gcol] = w_s
                gidx[gcol * 128 + part] = li_s
                # DRAM row of rank k: quad-interleaved within each 512-group
                zrow = nCB[r][h] * 512
                qq = np.arange(hsz)
                rk = pc["cpos"]
                quad = (rk // 512) * 512 + (rk % 128) * 4 + (rk // 128) % 4
                mrow = np.where(pc["cnt"] > 0, quad, zrow)
                midx[r * npad + h * hsz + qq] = mrow

        m = tshard == c
        tq = col[m] - c * sh
        wts = ew[m]
        o = np.argsort(tq, kind="stable")
        tq_s, w_s = tq[o], wts[o]
        slot = _slot_ranks(tq_s)
        tgrid = np.zeros((128, tcols), dtype=np.float32)
        tgrid[tq_s % 128, tofs[tq_s // 128] + slot] = w_s

        in_maps.append({
            "x_tab": x_tab,
            "x_own": np.ascontiguousarray(x_tab[c * npad:(c + 1) * npad]),
            "ew_grid": ew_grid,
            "tgrid": tgrid,
            "gidx": _wrap16(gidx),
            "midx": _wrap16(midx),
            "w1": np.asarray(W1, np.float32),
            "wcat": wcat,
            "b1row": np.asarray(b1, np.float32).reshape(1, -1),
            "bcatrow": bcat.reshape(1, -1),
            "ident": ident,
        })

    meta = dict(ncb=[[int(nCB[r][h]) for h in range(hh)] for r in range(nr)],
                S=[[list(map(int, S[r][h])) for h in range(hh)]
                   for r in range(nr)],
                TS=list(map(int, TS)), gcols=gcols, tcols=tcols,
                gofs={f"{r}_{h}": gofs[(r, h)] for r in range(nr)
                      for h in range(hh)})
    return in_maps, meta


# ----------------------------------------------------------------------------
# device program
# ----------------------------------------------------------------------------

def _build(cfg, meta, split=True):
    import concourse.bacc as bacc
    import concourse.mybir as mybir
    from concourse.tile import TileContext

    p, nb, npad, tbl = cfg["p"], cfg["nb"], cfg["npad"], cfg["tbl"]
    nr, rs, hh, hblk = cfg["nr"], cfg["rs"], cfg["hh"], cfg["hblk"]
    CPC, MBB, ACB, SG = cfg["cpc"], cfg["mbb"], cfg["acb"], cfg["sgrp"]
    SCLG = cfg["sclg"]
    ncb, S, TS = meta["ncb"], meta["S"], meta["TS"]
    gcols, tcols = meta["gcols"], meta["tcols"]
    gofs = {tuple(map(int, k.split("_"))): v for k, v in meta["gofs"].items()}
    f32, bf16, i16 = mybir.dt.float32, mybir.dt.bfloat16, mybir.dt.int16
    AX = mybir.AxisListType.X
    OP = mybir.AluOpType
    ACTF = mybir.ActivationFunctionType
    tgrp = rs // 512                        # 512-row groups per range (49)

    nc = bacc.Bacc(num_devices=p)
    x_tab = nc.declare_dram_parameter("x_tab", [tbl, 128], bf16, isOutput=False)
    x_own = nc.declare_dram_parameter("x_own", [npad, 128], bf16, isOutput=False)
    ew_grid = nc.declare_dram_parameter("ew_grid", [128, gcols], f32, isOutput=False)
    tgrid = nc.declare_dram_parameter("tgrid", [128, tcols], f32, isOutput=False)
    gidx = nc.declare_dram_parameter("gidx", [128, gcols * 8], i16, isOutput=False)
    midx = nc.declare_dram_parameter("midx", [128, nr * nb * 8], i16, isOutput=False)
    w1 = nc.declare_dram_parameter("w1", [128, 128], f32, isOutput=False)
    wcat = nc.declare_dram_parameter("wcat", [128, 128], f32, isOutput=False)
    b1row = nc.declare_dram_parameter("b1row", [1, 128], f32, isOutput=False)
    bcatrow = nc.declare_dram_parameter("bcatrow", [1, 128], f32, isOutput=False)
    ident = nc.declare_dram_parameter("ident", [128, 128], f32, isOutput=False)
    out_ext = nc.declare_dram_parameter("out", [npad, 128], f32, isOutput=True)

    with TileContext(nc) as tc:
        with tc.tile_pool(name="dram", bufs=1, space="DRAM") as dram, \
             tc.tile_pool(name="persist", bufs=1) as pp, \
             tc.tile_pool(name="scl", bufs=2) as sclp, \
             tc.tile_pool(name="grid", bufs=cfg["grid_bufs"]) as gp, \
             tc.tile_pool(name="acc", bufs=cfg["acc_bufs"]) as accp, \
             tc.tile_pool(name="mg", bufs=cfg["mg_bufs"]) as mgp, \
             tc.tile_pool(name="work", bufs=cfg["wp_bufs"]) as wp, \
             tc.tile_pool(name="psum", bufs=4, space="PSUM") as psp:

            xs_r = [dram.tile([rs, 128], bf16, tag=f"xs{r}", name=f"xs{r}")
                    for r in range(nr)]
            xs_own = dram.tile([npad, 128], bf16, tag="xso", name="xs_own")
            hs_shard = dram.tile([npad, 128], bf16, tag="hss", name="hs_shard")
            hs_tab = dram.tile([tbl, 128], bf16, tag="hst", name="hs_tab")
            dv_col = dram.tile([npad, 1], f32, tag="dvc", name="dv_col")
            dv_gat = dram.tile([tbl, 1], f32, tag="dvg", name="dv_gat")
            parts = {}
            for r in range(nr):
                for h in range(hh):
                    parts[(r, h)] = dram.tile(
                        [ncb[r][h] * 512 + 128, 128], bf16,
                        tag=f"pt{r}{h}", name=f"part{r}{h}")

            ewg_t = pp.tile([128, gcols], f32, tag="ewg", name="ewg_t")
            gidx_t = pp.tile([128, gcols * 8], i16, tag="gix", name="gidx_t")
            midx_t = pp.tile([128, nr * nb * 8], i16, tag="mix", name="midx_t")
            tg_t = pp.tile([128, tcols], f32, tag="tg", name="tg_t")
            w1_t = pp.tile([128, 128], f32, tag="w1", name="w1_t")
            wcat_t = pp.tile([128, 128], f32, tag="wc", name="wcat_t")
            b1_t = pp.tile([1, 128], f32, tag="b1", name="b1_t")
            bcat_t = pp.tile([1, 128], f32, tag="bc", name="bcat_t")
            id_t = pp.tile([128, 128], f32, tag="id", name="id_t")
            ones_t = pp.tile([1, 128], f32, tag="on", name="ones_t")
            deg_t = pp.tile([128, nb], f32, tag="dg", name="deg_t")
            d2_t = pp.tile([128, nb], f32, tag="d2", name="d2_t")
            dv_t = pp.tile([128, nb], f32, tag="dv", name="dv_t")
            dvq_t = pp.tile([128, (tbl // 512) * 4], f32, tag="dvq",
                            name="dvq_t")
            z_t = pp.tile([128, 128], bf16, tag="z", name="z_t")

            nc.sync.dma_start(out=tg_t[:], in_=tgrid[:])
            nc.vector.memset(ones_t[:], 1.0)
            nc.vector.memset(z_t[:], 0.0)

            # deg -> d2 (=1/deg) and dinv; own shard, block layout ----------
            tof = 0
            for b in range(nb):
                if TS[b] > 0:
                    nc.vector.tensor_reduce(deg_t[:, b:b + 1],
                                            tg_t[:, tof:tof + TS[b]],
                                            axis=AX, op=OP.add)
                else:
                    nc.vector.memset(deg_t[:, b:b + 1], 0.0)
                tof += TS[b]
            nc.vector.tensor_scalar_add(deg_t[:], deg_t[:], 1.0)
            nc.vector.reciprocal(d2_t[:], deg_t[:])
            nc.scalar.sqrt(dv_t[:], d2_t[:])
            nc.sync.dma_start(
                out=dv_col[:].rearrange("(b p) one -> p (b one)", p=128),
                in_=dv_t[:])

            groups = [list(range(p))]
            tc.strict_bb_all_engine_barrier()
            nc.gpsimd.collective_compute(
                "AllGather", OP.bypass, replica_groups=groups,
                ins=[dv_col.opt()], outs=[dv_gat.opt()])
            # bulk parameter loads stream while the dinv AllGather flies
            for t, src in [(ewg_t, ew_grid), (gidx_t, gidx), (midx_t, midx),
                           (w1_t, w1), (wcat_t, wcat), (b1_t, b1row),
                           (bcat_t, bcatrow), (id_t, ident)]:
                nc.sync.dma_start(out=t[:], in_=src[:])
            for r in range(nr):
                for h in range(hh):
                    nc.sync.dma_start(
                        out=parts[(r, h)][ncb[r][h] * 512:
                                          ncb[r][h] * 512 + 128, :],
                        in_=z_t[:])
            tc.strict_bb_all_engine_barrier()
            nc.sync.dma_start(
                out=dvq_t[:].rearrange("p (g j) -> p g j", j=4),
                in_=dv_gat[:].rearrange("(g p j) one -> p g (j one)",
                                        p=128, j=4))

            # xs_own = dinv * x_own (overlaps the start of the scale pass)
            for b0 in range(0, nb, SG):
                nbb = min(SG, nb - b0)
                xt = sclp.tile([128, SG * 128], bf16, tag="sxo", name="sxo")
                nc.sync.dma_start(
                    out=xt[:, :nbb * 128].rearrange("p (g c) -> p g c", c=128),
                    in_=x_own[b0 * 128:(b0 + nbb) * 128, :]
                    .rearrange("(g p) c -> p g c", p=128))
                for k in range(nbb):
                    nc.vector.tensor_scalar_mul(
                        xt[:, k * 128:(k + 1) * 128],
                        xt[:, k * 128:(k + 1) * 128],
                        dv_t[:, b0 + k:b0 + k + 1])
                nc.sync.dma_start(
                    out=xs_own[b0 * 128:(b0 + nbb) * 128, :]
                    .rearrange("(g p) c -> p g c", p=128),
                    in_=xt[:, :nbb * 128].rearrange("p (g c) -> p g c", c=128))

            # scale pass: xs_r = dinv * x_tab (bf16, 1KB descriptors) --------
            for r in range(nr):
                for g0 in range(0, tgrp, SCLG):
                    ngg = min(SCLG, tgrp - g0)
                    xt = sclp.tile([128, SCLG * 512], bf16, tag="sx", name="sx")
                    base = r * rs + g0 * 512
                    nc.sync.dma_start(
                        out=xt[:, :ngg * 512].rearrange(
                            "p (g jc) -> p g jc", jc=512),
                        in_=x_tab[base:base + ngg * 512, :]
                        .rearrange("(g p j) c -> p g (j c)", p=128, j=4))
                    for k in range(ngg):
                        gq = (r * tgrp + g0 + k) * 4
                        for j in range(4):
                            sl = xt[:, k * 512 + j * 128:k * 512 + (j + 1) * 128]
                            nc.vector.tensor_scalar_mul(
                                sl, sl, dvq_t[:, gq + j:gq + j + 1])
                    nc.sync.dma_start(
                        out=xs_r[r][g0 * 512:(g0 + ngg) * 512, :]
                        .rearrange("(g p j) c -> p g (j c)", p=128, j=4),
                        in_=xt[:, :ngg * 512].rearrange(
                            "p (g jc) -> p g jc", jc=512))

            # column -> compact block map per (r,h)
            cmap = {}
            for r in range(nr):
                for h in range(hh):
                    m = []
                    for sg in range(ncb[r][h] * 4):
                        m += [sg] * S[r][h][sg]
                    cmap[(r, h)] = m

            def aggregate(table_ap, r, h):
                colmap = cmap[(r, h)]
                Svec = S[r][h]
                total = len(colmap)
                part = parts[(r, h)]

                def flush(accb, CB):
                    nc.sync.dma_start(
                        out=part[CB * 512:(CB + 1) * 512, :]
                        .rearrange("(p j) c -> p (j c)", p=128),
                        in_=accb[:])

                done = 0
                acc = accb = None
                cur_cb = -1
                while done < total:
                    ncall = min(CPC, total - done)
                    grid = gp.tile([128, CPC * 128], bf16, tag="grid",
                                   name="grid")
                    go = gofs[(r, h)] + done
                    nc.gpsimd.dma_gather(
                        out_ap=grid[:, :ncall * 128].rearrange(
                            "p (g c) -> p g c", c=128),
                        in_ap=table_ap,
                        idxs_ap=gidx_t[:, go * 8:(go + ncall) * 8],
                        num_idxs=ncall * 128, num_idxs_reg=ncall * 128,
                        elem_size=128, single_packet=False)
                    for j in range(ncall):
                        lcol = done + j
                        sg = colmap[lcol]
                        CB, sj = sg // 4, sg % 4
                        if CB != cur_cb:
                            if accb is not None:
                                flush(accb, cur_cb)
                            acc = accp.tile([128, 512], f32, tag="acc",
                                            name="acc")
                            accb = accp.tile([128, 512], bf16, tag="accb",
                                             name="accb")
                            cur_cb = CB
                            if any(Svec[CB * 4 + q] == 0 for q in range(4)):
                                nc.vector.memset(accb[:], 0.0)
                        sl = acc[:, sj * 128:(sj + 1) * 128]
                        slb = accb[:, sj * 128:(sj + 1) * 128]
                        src = grid[:, j * 128:(j + 1) * 128]
                        sc = ewg_t[:, gofs[(r, h)] + lcol:gofs[(r, h)] + lcol + 1]
                        first = (lcol == 0 or colmap[lcol - 1] != sg)
                        last = (lcol + 1 == total or colmap[lcol + 1] != sg)
                        if first and last:
                            nc.vector.tensor_scalar_mul(slb, src, sc)
                        elif first:
                            nc.vector.tensor_scalar_mul(sl, src, sc)
                        elif last:
                            nc.vector.scalar_tensor_tensor(
                                slb, src, sc, sl, OP.mult, OP.add)
                        else:
                            nc.vector.scalar_tensor_tensor(
                                sl, src, sc, sl, OP.mult, OP.add)
                    done += ncall
                if accb is not None:
                    flush(accb, cur_cb)

            def merge_epilogue(h, own_src, front, wmat, bias_rhs,
                               actf, res_dt, res_writer, res_scale=None):
                b_lo = h * hblk
                for b0 in range(b_lo, b_lo + hblk, MBB):
                    nbb = min(MBB, b_lo + hblk - b0)
                    M = mgp.tile([128, nr * MBB * 128], bf16, tag="mg", name="M")
                    own = mgp.tile([128, MBB * 128], bf16, tag="own",
                                   name="own")
                    nc.sync.dma_start(
                        out=own[:, :nbb * 128].rearrange(
                            "p (g c) -> p g c", c=128),
                        in_=own_src[b0 * 128:(b0 + nbb) * 128, :]
                        .rearrange("(g p) c -> p g c", p=128))
                    for r in range(nr):
                        s0 = (r * nb + b0) * 128
                        nc.gpsimd.dma_gather(
                            out_ap=M[:, r * MBB * 128:(r * MBB + nbb) * 128]
                            .rearrange("p (g c) -> p g c", c=128),
                            in_ap=parts[(r, h)][:],
                            idxs_ap=midx_t[:, s0 // 16:(s0 + nbb * 128) // 16],
                            num_idxs=nbb * 128, num_idxs_reg=nbb * 128,
                            elem_size=128, single_packet=False)
                    for bi in range(nbb):
                        b = b0 + bi
                        def mg(r):
                            return M[:, (r * MBB + bi) * 128:
                                     (r * MBB + bi + 1) * 128]
                        s1 = wp.tile([128, 128], bf16, tag="s1", name="s1")
                        s2 = wp.tile([128, 128], bf16, tag="s2", name="s2")
                        nc.vector.tensor_tensor(s1[:], mg(0), mg(1), OP.add)
                        nc.vector.tensor_tensor(s2[:], mg(2), mg(3), OP.add)
                        s3 = wp.tile([128, 128], bf16, tag="s3", name="s3")
                        nc.vector.tensor_tensor(s3[:], s1[:], s2[:], OP.add)
                        agg = wp.tile([128, 128], f32, tag="agg", name="agg")
                        nc.vector.tensor_tensor(
                            agg[:], s3[:],
                            own[:, bi * 128:(bi + 1) * 128], OP.add)
                        asc = wp.tile([128, 128], f32, tag="asc", name="asc")
                        nc.scalar.activation(asc[:], agg[:], ACTF.Copy,
                                             scale=front[:, b:b + 1])
                        tps = psp.tile([128, 128], f32, tag="ps", name="tps")
                        nc.tensor.transpose(tps[:], asc[:], id_t[:])
                        aggT = wp.tile([128, 128], f32, tag="aggT", name="aggT")
                        nc.scalar.activation(aggT[:], tps[:], ACTF.Copy)
                        zps = psp.tile([128, 128], f32, tag="zps", name="zps")
                        nc.tensor.matmul(zps[:], ones_t[:], bias_rhs[:],
                                         start=True, stop=False)
                        nc.tensor.matmul(zps[:], aggT[:], wmat[:],
                                         start=False, stop=True)
                        res = wp.tile([128, 128], res_dt, tag=f"res{res_dt}",
                                      name="res")
                        if res_scale is not None:
                            nc.scalar.activation(res[:], zps[:], actf,
                                                 scale=res_scale[:, b:b + 1])
                        else:
                            nc.scalar.activation(res[:], zps[:], actf)
                        res_writer(b, res)

            # ---- layer 1 ----
            def l1_write(b, res):
                nc.sync.dma_start(out=hs_shard[b * 128:(b + 1) * 128, :],
                                  in_=res[:])

            for h in range(hh):
                for r in range(nr):
                    aggregate(xs_r[r][:], r, h)
                merge_epilogue(h, xs_own, dv_t, w1_t, b1_t,
                               ACTF.Relu, bf16, l1_write, res_scale=dv_t)

            # ---- AllGather hs ----
            if cfg["barriers"]:
                tc.strict_bb_all_engine_barrier()
            nc.gpsimd.collective_compute(
                "AllGather", OP.bypass, replica_groups=groups,
                ins=[hs_shard.opt()], outs=[hs_tab.opt()])
            if cfg["barriers"]:
                tc.strict_bb_all_engine_barrier()

            # ---- layers 2+3 ----
            def l2_write(b, res):
                nc.sync.dma_start(out=out_ext[b * 128:(b + 1) * 128, :],
                                  in_=res[:])

            for h in range(hh):
                for r in range(nr):
                    aggregate(hs_tab[r * rs:(r + 1) * rs, :], r, h)
                merge_epilogue(h, hs_shard, dv_t, wcat_t,
                               bcat_t, ACTF.Copy, f32, l2_write)

    nc.finalize()
    if split:
        _split_excess_waits(nc)
    return nc


# ----------------------------------------------------------------------------
# top-level entry
# ----------------------------------------------------------------------------

_CACHE = {}


def get_built(cfg, meta, split=True):
    key = repr((sorted(cfg.items()), repr(meta), split))
    if key not in _CACHE:
        _CACHE[key] = _build(cfg, meta, split=split)
    return _CACHE[key]


def run(inputs, cfg):
    from concourse.bass_utils import run_bass_kernel_spmd
    in_maps, meta = _preprocess(cfg, **inputs)
    nc = get_built(cfg, meta)
    res = run_bass_kernel_spmd(nc, in_maps, list(range(cfg["p"])))
    return postprocess(res.results, cfg)


def postprocess(results, cfg):
    n, sh, o, p = cfg["n"], cfg["sh"], cfg["o"], cfg["p"]
    mu = np.empty((n, o), dtype=np.float32)
    ls = np.empty((n, o), dtype=np.float32)
    for c in range(p):
        out = results[c]["out"]
        mu[c * sh:(c + 1) * sh] = out[:sh, :o]
        ls[c * sh:(c + 1) * sh] = out[:sh, o:2 * o]
    return mu, ls


def kernel(x, edge_index, edge_attr, W1, b1, Wmu, bmu, Wls, bls):
    return run(dict(x=x, edge_index=edge_index, edge_attr=edge_attr, W1=W1,
                    b1=b1, Wmu=Wmu, bmu=bmu, Wls=Wls, bls=bls), CFG)
